# revision 28
# baseline (speedup 1.0000x reference)
"""Trainium2 Bass kernel for nn_CAGpool (GNN message passing, CAG pooling).

Sharding: data-parallel over the 64 graph pairs -> 8 pairs (16 component
graphs of 512 nodes) per NeuronCore.  Message passing is dense matmul
against a per-graph 512x512 normalized adjacency (A+I with symmetric GCN
norm, a pure function of the edge-index structure) laid out host-side;
all compute on x/weights (GCN layers, attention pooling, top-k, pooled
conv, MLP) runs on device.

Per-core schedule: C/x stream in per-graph over SWDGE; the 3 GCN layers
run as a software-pipelined PE wavefront (the A-apply of item k-1 is
emitted under item k's PSUM->SBUF evacuation so the PE never stalls and
holds its fast p-state); the per-layer ReLU writebacks accumulate the
attention-pool means for free via activation accum_out.  Attention
scoring uses csel-selector matmuls whose selector builds sit on the
Scalar engine one step ahead of the PE.  The pooled-conv weight
precompute XWf = xcat @ Wf is mask-independent and is scheduled into the
otherwise PE-idle top-k window (top-k = 32 rounds of max8/match_replace
on Vector).  The pooled conv itself runs feat-major (4x 512-col matmuls
per graph); dropped nodes carry a zero dst-scale so their columns hold
relu(bf) exactly, which the mean pool corrects in closed form, and the
final attention weights are masked, so no per-column masking pass is
needed.
"""

import os
import numpy as np
import ml_dtypes

import concourse.bass as bass
import concourse.tile as tile
from concourse import bacc, mybir
from concourse.bass_utils import run_bass_kernel_spmd

F32 = mybir.dt.float32
BF16 = mybir.dt.bfloat16
F16 = mybir.dt.float16

NCORES = 8
B = 64
NPC = B // NCORES          # graph pairs per core (8)
NCG = 2 * NPC              # component graphs per core (16)
N = 512                    # nodes per component graph
K1 = 256
DEBUG = bool(int(os.environ.get("KERNEL_DEBUG", "0")))
STAGE = int(os.environ.get("KERNEL_STAGE", "4"))


def _layout(ent):
    offs, off = {}, 0
    for nm, w in ent:
        offs[nm] = (off, w)
        off += w
    return offs, off


WOFF, WF_TOT = _layout(
    [("W1", 128), ("W2", 128), ("W3", 128), ("Wgf", 128)]
    + [(f"Wg{i}", 384) for i in range(3)]
    + [(f"Wal{i}", 768) for i in range(6)]
    + [(f"Wf{i}", 128) for i in range(3)]
    + [("Wl1a", 128), ("Wl1b", 128), ("Wl2", 64), ("Wl3", 2),
       ("csel", 256), ("rsel", 2048)])
BOFF, BF_TOT = _layout(
    [("balcol", 6), ("bl1col", 1), ("bl2col", 1),
     ("bl3col", 1), ("identf", 128), ("bcols", 3), ("bfcol", 1)])


def _host_prep(inputs):
    """Per-core input maps. Integer index/count prep + dtype staging only."""
    x = np.asarray(inputs["x"], np.float32)

    s_loc, d_loc = {}, {}
    for comp, (sk, dk) in enumerate((("src_c1", "dst_c1"),
                                     ("src_c2", "dst_c2"))):
        base = (np.arange(B) * N)[:, None]
        s_loc[comp] = np.asarray(inputs[sk]).reshape(B, -1) - base
        d_loc[comp] = np.asarray(inputs[dk]).reshape(B, -1) - base

    in_maps = []
    for c in range(NCORES):
        xT = np.empty((128, NCG * N), ml_dtypes.bfloat16)
        cd = np.zeros((128, NCG * 2048), ml_dtypes.bfloat16)
        degr = np.empty((NCG, N), np.float32)
        for comp in range(2):
            for gl in range(NPC):
                g = c * NPC + gl
                cg = comp * NPC + gl
                r0 = g * 2 * N + comp * N
                xT[:, cg * N:(cg + 1) * N] = x[r0:r0 + N].T
                s = s_loc[comp][g].astype(np.int64)
                d = d_loc[comp][g].astype(np.int64)
                cnt = np.bincount(s * N + d, minlength=N * N)
                cmat = (cnt.reshape(N, N) + np.eye(N, dtype=np.int64)
                        ).astype(np.float32)
                dg = (np.bincount(d, minlength=N) + 1).astype(np.float32)
                degr[cg] = dg
                # symmetric gcn norm (graph-structure preprocessing)
                rsd = 1.0 / np.sqrt(dg)
                cmat *= rsd[:, None]
                cmat *= rsd[None, :]
                # [src, dst] -> [p=src%128, sblk*512 + dst]
                cd[:, cg * 2048:(cg + 1) * 2048] = (
                    cmat.reshape(4, 128, N).transpose(1, 0, 2)
                    .reshape(128, 2048))

        wpack = np.zeros((128, WF_TOT), np.float32)

        def put(nm, arr):
            o, w = WOFF[nm]
            arr = np.asarray(arr, np.float32)
            wpack[: arr.shape[0], o:o + arr.shape[1]] = arr

        put("W1", inputs["W1"]); put("W2", inputs["W2"]); put("W3", inputs["W3"])
        put("Wgf", inputs["Wg_fin"])
        for i in range(3):
            put(f"Wg{i}", np.asarray(inputs["Wg_att"])[i * 128:(i + 1) * 128])
        for i in range(6):
            put(f"Wal{i}", np.asarray(inputs["Wal"])[i * 128:(i + 1) * 128])
        for i in range(3):
            put(f"Wf{i}", np.asarray(inputs["Wf"])[i * 128:(i + 1) * 128])
        put("Wl1a", np.asarray(inputs["Wl1"])[:128])
        put("Wl1b", np.asarray(inputs["Wl1"])[128:])
        put("Wl2", inputs["Wl2"])
        put("Wl3", inputs["Wl3"])
        csel = np.zeros((128, 256), np.float32)
        for cg in range(NCG):
            csel[:, cg * 16 + cg] = 1.0
        put("csel", csel)
        rsel = np.zeros((16, 2048), np.float32)
        for cg in range(16):
            rsel[cg, cg * 128:(cg + 1) * 128] = 1.0
        put("rsel", rsel)

        bpack = np.zeros((128, BF_TOT), np.float32)

        def putb(nm, arr):
            o, w = BOFF[nm]
            arr = np.asarray(arr, np.float32)
            bpack[: arr.shape[0], o:o + arr.shape[1]] = arr

        putb("balcol", np.asarray(inputs["bal"]).reshape(6, 128).T)
        putb("bl1col", np.asarray(inputs["bl1"])[:, None])
        putb("bl2col", np.asarray(inputs["bl2"])[:, None])
        putb("bl3col", np.asarray(inputs["bl3"])[:, None])
        putb("bcols", np.stack([np.asarray(inputs["b1"]),
                                np.asarray(inputs["b2"]),
                                np.asarray(inputs["b3"])], 1))
        putb("identf", np.eye(128, dtype=np.float32))
        putb("bfcol", np.asarray(inputs["bf"])[:, None])

        in_maps.append({"xT": np.ascontiguousarray(xT),
                        "cd": np.ascontiguousarray(cd),
                        "degr": degr,
                        "wpack": wpack.astype(ml_dtypes.bfloat16),
                        "bpack": bpack})
    return in_maps


def _build():
    nc = bacc.Bacc("TRN2", target_bir_lowering=False, debug=False,
                   num_devices=NCORES)
    tin = {
        "xT": nc.dram_tensor("xT", [128, NCG * N], BF16, kind="ExternalInput"),
        "cd": nc.dram_tensor("cd", [128, NCG * 2048], BF16,
                             kind="ExternalInput"),
        "degr": nc.dram_tensor("degr", [NCG, N], F32, kind="ExternalInput"),
        "wpack": nc.dram_tensor("wpack", [128, WF_TOT], BF16,
                                kind="ExternalInput"),
        "bpack": nc.dram_tensor("bpack", [128, BF_TOT], F32,
                                kind="ExternalInput"),
    }
    t_out = nc.dram_tensor("out", [2, NPC], F32, kind="ExternalOutput")
    dbg = {}
    if DEBUG:
        for nm, shape, dt in (
                ("C", [128, NCG * 2048], BF16), ("deg", [16, N], F32),
                ("xcatT", [128, NCG * 1536], BF16), ("pvT", [128, 48], F32),
                ("scores", [16, N], F32), ("mask", [16, N], F32),
                ("alpha", [16, N], F32), ("gpT", [128, 48], F32),
                ("meanT", [128, 48], F32), ("hp", [128, NCG * 512], BF16)):
            dbg[nm] = nc.dram_tensor("dbg_" + nm, shape, dt,
                                     kind="ExternalOutput")
    with tile.TileContext(nc, linearize=bool(int(os.environ.get(
            "KERNEL_LINEARIZE", "0")))) as tc:
        _emit(nc, tc, tin, t_out, dbg)
    nc.compile()
    return nc


def _emit(nc, tc, tin, t_out, dbg):
    import contextlib
    ctx = contextlib.ExitStack()
    AX = mybir.AxisListType.X
    OP = mybir.AluOpType
    ACT = mybir.ActivationFunctionType

    const = ctx.enter_context(tc.tile_pool(name="const", bufs=1))
    rows = ctx.enter_context(tc.tile_pool(name="rows", bufs=1))
    work = ctx.enter_context(tc.tile_pool(name="work", bufs=3))
    scr = ctx.enter_context(tc.tile_pool(name="scr", bufs=3))
    ps_bc = ctx.enter_context(tc.tile_pool(name="psbc", bufs=2, space="PSUM"))
    ps_mm = ctx.enter_context(tc.tile_pool(name="psmm", bufs=4, space="PSUM"))
    ps_st = ctx.enter_context(tc.tile_pool(name="psst", bufs=1, space="PSUM"))
    ps_sm = ctx.enter_context(tc.tile_pool(name="pssm", bufs=1, space="PSUM"))

    def bigtile(pool, tag="mmw"):
        bt = pool.tile([128, 512], F32, tag=tag, name="bt")
        return bt

    wb = const.tile([128, WF_TOT], BF16, tag="wb")
    zeros512 = const.tile([128, 512], BF16, tag="zeros512")
    bp = const.tile([128, BF_TOT], F32, tag="bp")
    xTb = const.tile([128, NCG * N], BF16, tag="xTb")  # x -> xwf -> hp
    Call = const.tile([128, NCG * 2048], BF16, tag="Call")
    xcatT = const.tile([128, NCG * 1536], BF16, tag="xcatT")
    msqcolf = const.tile([128, 64], F32, tag="msqcolf")
    gqcol = const.tile([128, 64], F32, tag="gqcol")

    def W(nm):
        o, w = WOFF[nm]
        return wb[:, o:o + w]

    def Bc(nm):
        o, w = BOFF[nm]
        return bp[:, o:o + w]

    def csel(cg):
        o, _ = WOFF["csel"]
        return wb[:, o + cg * 16: o + (cg + 1) * 16]

    def rself(cg):
        o, _ = WOFF["rsel"]
        return wb[0:16, o + cg * 128: o + (cg + 1) * 128]

    identf = Bc("identf")

    def bcast_row(row_tile, cg, n):
        pb = ps_bc.tile([128, 512], F32, tag="bcast")
        nc.tensor.matmul(pb[:, :n], lhsT=rself(cg), rhs=row_tile[0:16, 0:n],
                         start=True, stop=True)
        return pb

    def tcol(dst_col4, row_tile, pool=rows):
        """Transpose a [16,512] f32 row into 4 [128,16] column groups."""
        for sblk in range(4):
            pt = ps_bc.tile([128, 512], F32, tag="bcast")
            nc.tensor.transpose(pt[:, 0:16],
                                row_tile[:, sblk * 128:(sblk + 1) * 128],
                                identf[0:16, 0:16])
            nc.vector.tensor_copy(dst_col4[:, sblk * 16:(sblk + 1) * 16],
                                  pt[:, 0:16])

    nc.vector.memset(zeros512[:], 0.0)

    # ---- input DMAs (small first, then per-cg C + x chunks) ---------------
    nc.sync.dma_start(bp[:], tin["bpack"].ap())
    degr = rows.tile([16, N], F32, tag="degr")
    nc.scalar.dma_start(degr[:], tin["degr"].ap())
    nc.gpsimd.dma_start(wb[:, 0:384], tin["wpack"].ap()[:, 0:384])
    nc.gpsimd.dma_start(xTb[:, 0:4 * N], tin["xT"].ap()[:, 0:4 * N])
    for cg in range(4):
        nc.gpsimd.dma_start(Call[:, cg * 2048:(cg + 1) * 2048],
                            tin["cd"].ap()[:, cg * 2048:(cg + 1) * 2048])
    for h in range(1, 4):
        c0, c1 = h * 4, h * 4 + 4
        nc.gpsimd.dma_start(xTb[:, c0 * N:c1 * N],
                            tin["xT"].ap()[:, c0 * N:c1 * N])
        nc.gpsimd.dma_start(Call[:, c0 * 2048:c1 * 2048],
                            tin["cd"].ap()[:, c0 * 2048:c1 * 2048])
    nc.gpsimd.dma_start(wb[:, 384:], tin["wpack"].ap()[:, 384:])

    if DEBUG:
        nc.sync.dma_start(dbg["deg"].ap(), degr[:])

    # ---- GCN wavefront ----------------------------------------------------
    items = [("gcn", l, cg) for l in range(3) for cg in range(NCG)]

    def key(it):
        kind, l, cg = it
        return 3.0 * cg + 0.5 + 8.25 * l

    items.sort(key=key)

    meanT = rows.tile([128, 48], F32, tag="meanT")

    def emit_apply(l, cg, xws):
        ph = ps_mm.tile([128, 512], F32, tag="mmw")
        for sblk in range(4):
            nc.tensor.matmul(
                ph[:],
                lhsT=xws[:, sblk * 128:(sblk + 1) * 128],
                rhs=Call[:, cg * 2048 + sblk * 512:
                         cg * 2048 + (sblk + 1) * 512],
                start=(sblk == 0), stop=(sblk == 3))
        xout = xcatT[:, cg * 1536 + l * 512: cg * 1536 + (l + 1) * 512]
        mcol = meanT[:, l * 16 + cg: l * 16 + cg + 1]
        if (l + cg) % 2 == 0:
            nc.scalar.activation(xout, ph[:], ACT.Relu,
                                 bias=Bc("bcols")[:, l:l + 1],
                                 accum_out=mcol)
        else:
            nc.vector.scalar_tensor_tensor(
                xout, ph[:], Bc("bcols")[:, l:l + 1], zeros512[:],
                op0=OP.add, op1=OP.max, accum_out=mcol)

    pending = None
    for kind, l, cg in items:
        if l < 3:
            wl = W(("W1", "W2", "W3")[l])
            xws = work.tile([128, 512], BF16, tag="xws")
            pxw = ps_mm.tile([128, 512], F32, tag="mmw")
            for nt in range(4):
                if l == 0:
                    lhsT = xTb[:, cg * N + nt * 128: cg * N + (nt + 1) * 128]
                else:
                    lhsT = xcatT[:, cg * 1536 + (l - 1) * 512 + nt * 128:
                                 cg * 1536 + (l - 1) * 512 + (nt + 1) * 128]
                nc.tensor.matmul(pxw[:, nt * 128:(nt + 1) * 128], lhsT=lhsT,
                                 rhs=wl, start=True, stop=True)
            nc.vector.tensor_copy(xws[:], pxw[:])
            # software pipeline: emit the A-apply of the PREVIOUS item so
            # the PE never waits on this item's PSUM->SBUF copy
            if pending is not None:
                emit_apply(*pending)
            pending = (l, cg, xws)
    if pending is not None:
        emit_apply(*pending)

    def emit_xwf(cg):
        # XWf = xcat @ Wf for all nodes (pre-mask), node-major
        pxp = ps_mm.tile([128, 512], F32, tag="mmw", name="pxp")
        for nt in range(4):
            for ci in range(3):
                nc.tensor.matmul(
                    pxp[:, nt * 128:(nt + 1) * 128],
                    lhsT=xcatT[:, cg * 1536 + ci * 512 + nt * 128:
                               cg * 1536 + ci * 512 + (nt + 1) * 128],
                    rhs=W(f"Wf{ci}"), start=(ci == 0), stop=(ci == 2))
        nc.scalar.activation(xTb[:, cg * N:(cg + 1) * N], pxp[:], ACT.Copy)
    xwf = xTb
    if DEBUG:
        nc.sync.dma_start(dbg["xcatT"].ap(), xcatT[:])
        nc.sync.dma_start(dbg["C"].ap(), Call[:])

    if STAGE < 2:
        o3 = rows.tile([2, NPC], F32, tag="o3")
        nc.vector.memset(o3[:], 0.0)
        nc.sync.dma_start(t_out.ap(), o3[:])
        ctx.close()
        return

    # ---- attention pool (cT -> alpha -> gp); mean accumulated in-layer ----
    meanTb = rows.tile([128, 48], BF16, tag="meanTb")
    nc.scalar.activation(meanTb[:], meanT[:], ACT.Copy, scale=1.0 / N)
    if DEBUG:
        nc.sync.dma_start(dbg["meanT"].ap(), meanT[:])

    for cg in range(4):
        emit_xwf(cg)
    cT = rows.tile([128, 48], F32, tag="cT")
    for fo in range(3):
        pc = ps_sm.tile([128, 16], F32, tag="s16")
        for fi in range(3):
            nc.tensor.matmul(pc[:],
                             lhsT=W(f"Wg{fi}")[:, fo * 128:(fo + 1) * 128],
                             rhs=meanTb[:, fi * 16:(fi + 1) * 16],
                             start=(fi == 0), stop=(fi == 2))
        nc.scalar.activation(cT[:, fo * 16:(fo + 1) * 16], pc[:], ACT.Tanh)

    ps_al = ps_st.tile([16, N], F32, tag="stat")
    alq = []
    for cg in range(NCG):
        for ch in range(3):
            mlh = work.tile([128, 16], BF16, tag="mlh")
            nc.scalar.activation(mlh[:], csel(cg), ACT.Copy,
                                 scale=cT[:, ch * 16 + cg: ch * 16 + cg + 1])
            alq.append((mlh, cg, ch))
            if len(alq) > 1:
                m0, c0, h0 = alq.pop(0)
                nc.tensor.matmul(
                    ps_al[:], lhsT=m0[:],
                    rhs=xcatT[:, c0 * 1536 + h0 * 512:
                              c0 * 1536 + (h0 + 1) * 512],
                    start=(c0 == 0 and h0 == 0), stop=False)
    m0, c0, h0 = alq.pop(0)
    nc.tensor.matmul(
        ps_al[:], lhsT=m0[:],
        rhs=xcatT[:, c0 * 1536 + h0 * 512: c0 * 1536 + (h0 + 1) * 512],
        start=False, stop=True)
    alpha_row = rows.tile([16, N], BF16, tag="alpha")
    nc.scalar.activation(alpha_row[:], ps_al[:], ACT.Sigmoid)
    for cg in range(4, 6):
        emit_xwf(cg)
    if DEBUG:
        alpha_f = rows.tile([16, N], F32, tag="alphaf")
        nc.vector.tensor_copy(alpha_f[:], alpha_row[:])
        nc.sync.dma_start(dbg["alpha"].ap(), alpha_f[:])

    gpT = rows.tile([128, 48], F32, tag="gpT")
    for cg in range(NCG):
        pab = bcast_row(alpha_row, cg, N)
        for ch in range(3):
            sc = scr.tile([128, 512], BF16, tag="scr")
            nc.vector.scalar_tensor_tensor(
                sc[:], xcatT[:, cg * 1536 + ch * 512: cg * 1536 + (ch + 1) * 512],
                1.0, pab[:], op0=OP.mult, op1=OP.mult,
                accum_out=gpT[:, ch * 16 + cg: ch * 16 + cg + 1])
    if DEBUG:
        nc.sync.dma_start(dbg["gpT"].ap(), gpT[:])

    # ---- att_lin: pv = [gp1, gp2] @ Wal + bal -----------------------------
    gpcatTb = rows.tile([128, 48], BF16, tag="gpcatTb")
    for j in range(6):
        comp, ch = j // 3, j % 3
        nc.vector.tensor_copy(
            gpcatTb[:, j * 8:(j + 1) * 8],
            gpT[:, ch * 16 + comp * 8: ch * 16 + comp * 8 + 8])
    pvTb = rows.tile([128, 48], BF16, tag="pvTb")
    pvTf = rows.tile([128, 48], F32, tag="pvTf")
    for co in range(6):
        pp = ps_sm.tile([128, 16], F32, tag="s16")
        for ci in range(6):
            nc.tensor.matmul(pp[:, 0:8],
                             lhsT=W(f"Wal{ci}")[:, co * 128:(co + 1) * 128],
                             rhs=gpcatTb[:, ci * 8:(ci + 1) * 8],
                             start=(ci == 0), stop=(ci == 5))
        nc.vector.tensor_scalar(pvTf[:, co * 8:(co + 1) * 8], pp[:, 0:8],
                                Bc("balcol")[:, co:co + 1], None, op0=OP.add)
        nc.vector.tensor_copy(pvTb[:, co * 8:(co + 1) * 8],
                              pvTf[:, co * 8:(co + 1) * 8])
    if DEBUG:
        nc.sync.dma_start(dbg["pvT"].ap(), pvTf[:])

    # ---- ||pv|| then scores ----------------------------------------------
    rsncol = rows.tile([16, 1], F32, tag="rsncol")
    pn = ps_sm.tile([128, 16], F32, tag="s16")
    for ci in range(6):
        comp = ci // 3
        mpv = work.tile([128, 16], BF16, tag="mlh")
        nc.vector.memset(mpv[:], 0.0)
        nc.vector.tensor_copy(mpv[:, comp * 8:(comp + 1) * 8],
                              pvTb[:, ci * 8:(ci + 1) * 8])
        nc.tensor.matmul(pn[0:16, :], lhsT=mpv[:], rhs=mpv[:],
                         start=(ci == 0), stop=(ci == 5))
    dd = rows.tile([16, 16], F32, tag="dd")
    nc.vector.tensor_tensor(dd[:], pn[0:16, :], identf[0:16, 0:16],
                            op=OP.mult)
    nn = rows.tile([16, 1], F32, tag="nn")
    nc.vector.tensor_reduce(nn[:], dd[:], axis=AX, op=OP.add)
    sqn = rows.tile([16, 1], F32, tag="sqn")
    nc.scalar.activation(sqn[:], nn[:], ACT.Sqrt)
    nc.vector.reciprocal_approx_fast(rsncol[:], sqn[:])

    ps_sc = ps_st.tile([16, N], F32, tag="stat")
    scq = []
    for cg in range(NCG):
        comp, g = cg // NPC, cg % NPC
        for ci in range(3):
            mlh = work.tile([128, 16], BF16, tag="mlh")
            nc.scalar.activation(
                mlh[:], csel(cg), ACT.Copy,
                scale=pvTf[:, (comp * 3 + ci) * 8 + g:
                           (comp * 3 + ci) * 8 + g + 1])
            scq.append((mlh, cg, ci))
            if len(scq) > 1:
                m0, c0, h0 = scq.pop(0)
                nc.tensor.matmul(
                    ps_sc[:], lhsT=m0[:],
                    rhs=xcatT[:, c0 * 1536 + h0 * 512:
                              c0 * 1536 + (h0 + 1) * 512],
                    start=(c0 == 0 and h0 == 0), stop=False)
    m0, c0, h0 = scq.pop(0)
    nc.tensor.matmul(
        ps_sc[:], lhsT=m0[:],
        rhs=xcatT[:, c0 * 1536 + h0 * 512: c0 * 1536 + (h0 + 1) * 512],
        start=False, stop=True)
    score_row = rows.tile([16, N], F32, tag="score")
    nc.scalar.activation(score_row[:], ps_sc[:], ACT.Copy, scale=rsncol[:])
    score16 = rows.tile([16, N], F16, tag="score16")
    nc.vector.tensor_copy(score16[:], score_row[:])
    for cg in range(6, NCG):
        emit_xwf(cg)
    if DEBUG:
        nc.sync.dma_start(dbg["scores"].ap(), score_row[:])

    if STAGE < 3:
        o3 = rows.tile([2, NPC], F32, tag="o3")
        nc.vector.memset(o3[:], 0.0)
        nc.sync.dma_start(t_out.ap(), o3[:])
        ctx.close()
        return

    # ---- top-256 mask (32 rounds of fp16 max8 + match_replace) ------------
    # match_replace is positional, so fp16 ties still yield exactly K1 kept
    # positions; a tie can only swap nodes whose scores differ by < 1 fp16
    # ulp, which is well inside the accuracy budget.
    cur = rows.tile([16, N], F16, tag="cur")
    nc.vector.tensor_copy(cur[:], score16[:])
    mx = rows.tile([16, 8], F16, tag="mx")
    for _ in range(K1 // 8):
        nc.vector.max(out=mx[:], in_=cur[:])
        nc.vector.match_replace(out=cur[:], in_to_replace=mx[:],
                                in_values=cur[:], imm_value=-60000.0)
    mask_row = rows.tile([16, N], F32, tag="mask")
    nc.vector.tensor_tensor(mask_row[:], score16[:], cur[:], op=OP.not_equal)
    if DEBUG:
        nc.sync.dma_start(dbg["mask"].ap(), mask_row[:])
    sig_row = rows.tile([16, N], F32, tag="sig")
    nc.scalar.activation(sig_row[:], score_row[:], ACT.Sigmoid)

    sq_row = rows.tile([16, N], F32, tag="sq")
    nc.scalar.activation(sq_row[:], degr[:], ACT.Sqrt)
    msq_row = rows.tile([16, N], F32, tag="msq")
    nc.vector.tensor_tensor(msq_row[:], mask_row[:], sq_row[:], op=OP.mult)
    tcol(msqcolf, msq_row)

    # ---- pooled degree ----------------------------------------------------
    if STAGE < 4:
        o3 = rows.tile([2, NPC], F32, tag="o3")
        nc.vector.memset(o3[:], 0.0)
        nc.sync.dma_start(t_out.ap(), o3[:])
        ctx.close()
        return
    ps_d2 = ps_st.tile([16, N], F32, tag="stat")
    d2q = []
    for cg in range(NCG):
        for sblk in range(4):
            mlh = work.tile([128, 16], BF16, tag="mlh")
            mcol = msqcolf[:, sblk * 16 + cg: sblk * 16 + cg + 1]
            if sblk % 2 == 0:
                nc.scalar.activation(mlh[:], csel(cg), ACT.Copy, scale=mcol)
            else:
                nc.vector.tensor_scalar(mlh[:], csel(cg), mcol, None,
                                        op0=OP.mult)
            d2q.append((mlh, cg, sblk))
            if len(d2q) > 1:
                m0, c0, s0 = d2q.pop(0)
                nc.tensor.matmul(
                    ps_d2[:], lhsT=m0[:],
                    rhs=Call[:, c0 * 2048 + s0 * 512:
                             c0 * 2048 + (s0 + 1) * 512],
                    start=(c0 == 0 and s0 == 0), stop=False)
    m0, c0, s0 = d2q.pop(0)
    nc.tensor.matmul(
        ps_d2[:], lhsT=m0[:],
        rhs=Call[:, c0 * 2048 + s0 * 512: c0 * 2048 + (s0 + 1) * 512],
        start=False, stop=True)
    deg2_row = rows.tile([16, N], F32, tag="deg2")
    nc.vector.tensor_tensor(deg2_row[:], ps_d2[:], msq_row[:], op=OP.mult)
    nc.vector.tensor_tensor(deg2_row[:], deg2_row[:], mask_row[:],
                            op=OP.subtract)
    nc.vector.tensor_scalar(deg2_row[:], deg2_row[:], 1.0, None, op0=OP.add)
    sq2_row = rows.tile([16, N], F32, tag="sq2")
    nc.scalar.activation(sq2_row[:], deg2_row[:], ACT.Sqrt)
    rsd2_row = rows.tile([16, N], F32, tag="rsd2")
    nc.vector.reciprocal_approx_fast(rsd2_row[:], sq2_row[:])
    q_row = rows.tile([16, N], F32, tag="qrow")
    nc.vector.tensor_tensor(q_row[:], rsd2_row[:], msq_row[:], op=OP.mult)
    q_rowb = rows.tile([16, N], BF16, tag="qrowb")
    nc.vector.tensor_copy(q_rowb[:], q_row[:])
    gq_row = rows.tile([16, N], F32, tag="gqrow")
    nc.vector.scalar_tensor_tensor(gq_row[:], sig_row[:], 1.0, q_row[:],
                                   op0=OP.mult, op1=OP.mult)
    tcol(gqcol, gq_row)

    # ---- pooled conv (feat-major) + corrected mean pool -------------------
    # z[f,d] = sum_s C[s,d] gq_s xwf[s,f]; hp = relu(q_d z + bf).
    # Dropped dst cols have q_d = 0 so hp = relu(bf) there; the mean is
    # corrected by subtracting exactly (N-K1) relu(bf) per row, and the
    # final attention weights are masked, so those columns never leak.
    rbf256 = rows.tile([128, 1], F32, tag="rbf256")
    nc.scalar.activation(rbf256[:], Bc("bfcol"), ACT.Relu, scale=float(N - K1))
    rawsum = rows.tile([128, 16], F32, tag="rawsum")

    def emit_xwps(cg):
        xwps = work.tile([128, 512], BF16, tag="xws", name="xwps")
        for nt in range(4):
            sl_in = xwf[:, cg * N + nt * 128: cg * N + (nt + 1) * 128]
            sl_out = xwps[:, nt * 128:(nt + 1) * 128]
            gcol = gqcol[:, nt * 16 + cg: nt * 16 + cg + 1]
            nc.vector.tensor_scalar(sl_out, sl_in, gcol, None,
                                    op0=OP.mult)
        return xwps

    xwps_q = [emit_xwps(0)]
    for cg in range(NCG):
        if cg + 1 < NCG:
            xwps_q.append(emit_xwps(cg + 1))
        xwps = xwps_q.pop(0)
        z = ps_mm.tile([128, 512], F32, tag="mmw")
        for sblk in range(4):
            nc.tensor.matmul(
                z[:],
                lhsT=xwps[:, sblk * 128:(sblk + 1) * 128],
                rhs=Call[:, cg * 2048 + sblk * 512:
                         cg * 2048 + (sblk + 1) * 512],
                start=(sblk == 0), stop=(sblk == 3))
        bq = bcast_row(q_rowb, cg, N)
        bqs = scr.tile([128, 512], BF16, tag="scr")
        if cg % 2 == 0:
            nc.scalar.activation(bqs[:], bq[:], ACT.Copy)
        else:
            nc.vector.tensor_copy(bqs[:], bq[:])
        nc.vector.tensor_tensor(z[:], z[:], bqs[:], op=OP.mult)
        hp = xwf[:, cg * N:(cg + 1) * N]
        nc.scalar.activation(hp, z[:], ACT.Relu, bias=Bc("bfcol")[:, 0:1],
                             accum_out=rawsum[:, cg:cg + 1])
    hpall = xwf
    if DEBUG:
        nc.sync.dma_start(dbg["hp"].ap(), hpall[:])

    # ---- final attention pool (feat-major) --------------------------------
    mT2b = rows.tile([128, 16], BF16, tag="mT2b")
    nc.vector.tensor_scalar(mT2b[:], rawsum[:], rbf256[:, 0:1], 1.0 / K1,
                            op0=OP.subtract, op1=OP.mult)
    pc2 = ps_sm.tile([128, 16], F32, tag="s16")
    nc.tensor.matmul(pc2[:], lhsT=W("Wgf"), rhs=mT2b[:], start=True,
                     stop=True)
    c2Tf = rows.tile([128, 16], F32, tag="c2Tf")
    nc.scalar.activation(c2Tf[:], pc2[:], ACT.Tanh)

    ps_a2 = ps_st.tile([16, N], F32, tag="stat")
    a2q = []
    for cg in range(NCG):
        mlh = work.tile([128, 16], BF16, tag="mlh")
        nc.scalar.activation(mlh[:], csel(cg), ACT.Copy,
                             scale=c2Tf[:, cg:cg + 1])
        a2q.append((mlh, cg))
        if len(a2q) > 1:
            m0, c0 = a2q.pop(0)
            nc.tensor.matmul(ps_a2[:], lhsT=m0[:],
                             rhs=hpall[:, c0 * N:(c0 + 1) * N],
                             start=(c0 == 0), stop=False)
    m0, c0 = a2q.pop(0)
    nc.tensor.matmul(ps_a2[:], lhsT=m0[:], rhs=hpall[:, c0 * N:(c0 + 1) * N],
                     start=False, stop=True)
    wsum_row = rows.tile([16, N], F32, tag="wsum")
    nc.scalar.activation(wsum_row[:], ps_a2[:], ACT.Sigmoid)
    wsum_rowb = rows.tile([16, N], BF16, tag="wsumb")
    nc.vector.tensor_tensor(wsum_rowb[:], wsum_row[:], mask_row[:],
                            op=OP.mult)

    gcat = rows.tile([128, 16], F32, tag="gcat")
    for cg in range(NCG):
        bw = bcast_row(wsum_rowb, cg, N)
        sc3 = scr.tile([128, 512], BF16, tag="scr")
        nc.vector.scalar_tensor_tensor(
            sc3[:], hpall[:, cg * N:(cg + 1) * N], 1.0, bw[:],
            op0=OP.mult, op1=OP.mult, accum_out=gcat[:, cg:cg + 1])

    # ---- final MLP --------------------------------------------------------
    pcat = rows.tile([128, 16], BF16, tag="pcat")
    nc.vector.tensor_copy(pcat[:], gcat[:])
    p1b = bigtile(ps_mm)
    p1 = p1b[:, 0:128]
    nc.tensor.matmul(p1[:, 0:NPC], lhsT=W("Wl1a"), rhs=pcat[:, 0:NPC],
                     start=True, stop=False)
    nc.tensor.matmul(p1[:, 0:NPC], lhsT=W("Wl1b"), rhs=pcat[:, NPC:2 * NPC],
                     start=False, stop=True)
    o1 = rows.tile([128, NPC], BF16, tag="o1")
    nc.scalar.activation(o1[:], p1[:, 0:NPC], ACT.Relu, bias=Bc("bl1col")[:])
    p2b = bigtile(ps_mm)
    p2 = p2b[:, 0:128]
    nc.tensor.matmul(p2[0:64, 0:NPC], lhsT=W("Wl2"), rhs=o1[:], start=True,
                     stop=True)
    o2 = rows.tile([64, NPC], BF16, tag="o2")
    nc.scalar.activation(o2[:], p2[0:64, 0:NPC], ACT.Relu,
                         bias=Bc("bl2col")[0:64, :])
    p3b = bigtile(ps_mm)
    p3 = p3b[:, 0:128]
    nc.tensor.matmul(p3[0:2, 0:NPC], lhsT=W("Wl3")[0:64, :], rhs=o2[:],
                     start=True, stop=True)
    o3 = rows.tile([2, NPC], F32, tag="o3")
    nc.vector.tensor_scalar(o3[:], p3[0:2, 0:NPC], Bc("bl3col")[0:2, :],
                            None, op0=OP.add)
    nc.sync.dma_start(t_out.ap(), o3[:])
    ctx.close()


_NC_CACHE = {}


def _get_nc():
    key = (STAGE, DEBUG)
    if key not in _NC_CACHE:
        _NC_CACHE[key] = _build()
    return _NC_CACHE[key]


def kernel(**inputs):
    in_maps = _host_prep(inputs)
    nc = _get_nc()
    trace = bool(int(os.environ.get("KERNEL_TRACE", "0")))
    tmpdir = os.environ.get("KERNEL_TRACE_DIR") or None
    res = run_bass_kernel_spmd(nc, in_maps, core_ids=list(range(NCORES)),
                               trace=trace, tmpdir=tmpdir)
    out = np.empty((B, 2), np.float32)
    for c in range(NCORES):
        out[c * NPC:(c + 1) * NPC] = res.results[c]["out"].T
    kernel._last = res
    return out


# revision 29
# speedup vs baseline: 1.1705x; 1.1705x over previous
"""Trainium2 Bass kernel for nn_CAGpool (GNN message passing, CAG pooling).

Sharding: data-parallel over the 64 graph pairs -> 8 pairs (16 component
graphs of 512 nodes) per NeuronCore.  Message passing is dense matmul
against a per-graph 512x512 normalized adjacency (A+I with symmetric GCN
norm, a pure function of the edge-index structure) laid out host-side;
all compute on x/weights (GCN layers, attention pooling, top-k, pooled
conv, MLP) runs on device.

Per-core schedule: C/x stream in per-graph over SWDGE; the 3 GCN layers
run as a software-pipelined PE wavefront (the A-apply of item k-1 is
emitted under item k's PSUM->SBUF evacuation so the PE never stalls and
holds its fast p-state); the per-layer ReLU writebacks accumulate the
attention-pool means for free via activation accum_out.  Attention
scoring uses csel-selector matmuls whose selector builds sit on the
Scalar engine one step ahead of the PE.  The pooled-conv weight
precompute XWf = xcat @ Wf is mask-independent and is scheduled into the
otherwise PE-idle top-k window (top-k = 32 rounds of max8/match_replace
on Vector).  The pooled conv itself runs feat-major (4x 512-col matmuls
per graph); dropped nodes carry a zero dst-scale so their columns hold
relu(bf) exactly, which the mean pool corrects in closed form, and the
final attention weights are masked, so no per-column masking pass is
needed.
"""

import os
import numpy as np
import ml_dtypes

import concourse.bass as bass
import concourse.tile as tile
from concourse import bacc, mybir
from concourse.bass_utils import run_bass_kernel_spmd

F32 = mybir.dt.float32
BF16 = mybir.dt.bfloat16
F16 = mybir.dt.float16

NCORES = 8
B = 64
NPC = B // NCORES          # graph pairs per core (8)
NCG = 2 * NPC              # component graphs per core (16)
N = 512                    # nodes per component graph
K1 = 256
DEBUG = bool(int(os.environ.get("KERNEL_DEBUG", "0")))
STAGE = int(os.environ.get("KERNEL_STAGE", "4"))


def _layout(ent):
    offs, off = {}, 0
    for nm, w in ent:
        offs[nm] = (off, w)
        off += w
    return offs, off


WOFF, WF_TOT = _layout(
    [("W1", 128), ("W2", 128), ("W3", 128), ("Wgf", 128)]
    + [(f"Wg{i}", 384) for i in range(3)]
    + [(f"Wal{i}", 768) for i in range(6)]
    + [(f"Wf{i}", 128) for i in range(3)]
    + [("Wl1a", 128), ("Wl1b", 128), ("Wl2", 64), ("Wl3", 2),
       ("csel", 256), ("rsel", 2048)])
BOFF, BF_TOT = _layout(
    [("balcol", 6), ("bl1col", 1), ("bl2col", 1),
     ("bl3col", 1), ("identf", 128), ("bcols", 3), ("bfcol", 1)])


def _host_prep(inputs):
    """Per-core input maps. Integer index/count prep + dtype staging only."""
    x = np.asarray(inputs["x"], np.float32)

    s_loc, d_loc = {}, {}
    for comp, (sk, dk) in enumerate((("src_c1", "dst_c1"),
                                     ("src_c2", "dst_c2"))):
        base = (np.arange(B) * N)[:, None]
        s_loc[comp] = np.asarray(inputs[sk]).reshape(B, -1) - base
        d_loc[comp] = np.asarray(inputs[dk]).reshape(B, -1) - base

    in_maps = []
    for c in range(NCORES):
        xT = np.empty((128, NCG * N), ml_dtypes.bfloat16)
        cd = np.zeros((128, NCG * 2048), ml_dtypes.bfloat16)
        degr = np.empty((NCG, N), np.float32)
        for comp in range(2):
            for gl in range(NPC):
                g = c * NPC + gl
                cg = comp * NPC + gl
                r0 = g * 2 * N + comp * N
                xT[:, cg * N:(cg + 1) * N] = x[r0:r0 + N].T
                s = s_loc[comp][g].astype(np.int64)
                d = d_loc[comp][g].astype(np.int64)
                cnt = np.bincount(s * N + d, minlength=N * N)
                cmat = (cnt.reshape(N, N) + np.eye(N, dtype=np.int64)
                        ).astype(np.float32)
                dg = (np.bincount(d, minlength=N) + 1).astype(np.float32)
                degr[cg] = dg
                # symmetric gcn norm (graph-structure preprocessing)
                rsd = 1.0 / np.sqrt(dg)
                cmat *= rsd[:, None]
                cmat *= rsd[None, :]
                # [src, dst] -> [p=src%128, sblk*512 + dst]
                cd[:, cg * 2048:(cg + 1) * 2048] = (
                    cmat.reshape(4, 128, N).transpose(1, 0, 2)
                    .reshape(128, 2048))

        wpack = np.zeros((128, WF_TOT), np.float32)

        def put(nm, arr):
            o, w = WOFF[nm]
            arr = np.asarray(arr, np.float32)
            wpack[: arr.shape[0], o:o + arr.shape[1]] = arr

        put("W1", inputs["W1"]); put("W2", inputs["W2"]); put("W3", inputs["W3"])
        put("Wgf", inputs["Wg_fin"])
        for i in range(3):
            put(f"Wg{i}", np.asarray(inputs["Wg_att"])[i * 128:(i + 1) * 128])
        for i in range(6):
            put(f"Wal{i}", np.asarray(inputs["Wal"])[i * 128:(i + 1) * 128])
        for i in range(3):
            put(f"Wf{i}", np.asarray(inputs["Wf"])[i * 128:(i + 1) * 128])
        put("Wl1a", np.asarray(inputs["Wl1"])[:128])
        put("Wl1b", np.asarray(inputs["Wl1"])[128:])
        put("Wl2", inputs["Wl2"])
        put("Wl3", inputs["Wl3"])
        csel = np.zeros((128, 256), np.float32)
        for cg in range(NCG):
            csel[:, cg * 16 + cg] = 1.0
        put("csel", csel)
        rsel = np.zeros((16, 2048), np.float32)
        for cg in range(16):
            rsel[cg, cg * 128:(cg + 1) * 128] = 1.0
        put("rsel", rsel)

        bpack = np.zeros((128, BF_TOT), np.float32)

        def putb(nm, arr):
            o, w = BOFF[nm]
            arr = np.asarray(arr, np.float32)
            bpack[: arr.shape[0], o:o + arr.shape[1]] = arr

        putb("balcol", np.asarray(inputs["bal"]).reshape(6, 128).T)
        putb("bl1col", np.asarray(inputs["bl1"])[:, None])
        putb("bl2col", np.asarray(inputs["bl2"])[:, None])
        putb("bl3col", np.asarray(inputs["bl3"])[:, None])
        putb("bcols", np.stack([np.asarray(inputs["b1"]),
                                np.asarray(inputs["b2"]),
                                np.asarray(inputs["b3"])], 1))
        putb("identf", np.eye(128, dtype=np.float32))
        putb("bfcol", np.asarray(inputs["bf"])[:, None])

        in_maps.append({"xT": np.ascontiguousarray(xT),
                        "cd": np.ascontiguousarray(cd),
                        "degr": degr,
                        "wpack": wpack.astype(ml_dtypes.bfloat16),
                        "bpack": bpack})
    return in_maps


def _build():
    nc = bacc.Bacc("TRN2", target_bir_lowering=False, debug=False,
                   num_devices=NCORES)
    tin = {
        "xT": nc.dram_tensor("xT", [128, NCG * N], BF16, kind="ExternalInput"),
        "cd": nc.dram_tensor("cd", [128, NCG * 2048], BF16,
                             kind="ExternalInput"),
        "degr": nc.dram_tensor("degr", [NCG, N], F32, kind="ExternalInput"),
        "wpack": nc.dram_tensor("wpack", [128, WF_TOT], BF16,
                                kind="ExternalInput"),
        "bpack": nc.dram_tensor("bpack", [128, BF_TOT], F32,
                                kind="ExternalInput"),
    }
    t_out = nc.dram_tensor("out", [2, NPC], F32, kind="ExternalOutput")
    dbg = {}
    if DEBUG:
        for nm, shape, dt in (
                ("C", [128, NCG * 2048], BF16), ("deg", [16, N], F32),
                ("xcatT", [128, NCG * 1536], BF16), ("pvT", [128, 48], F32),
                ("scores", [16, N], F32), ("mask", [16, N], F32),
                ("alpha", [16, N], F32), ("gpT", [128, 48], F32),
                ("meanT", [128, 48], F32), ("hp", [128, NCG * 512], BF16)):
            dbg[nm] = nc.dram_tensor("dbg_" + nm, shape, dt,
                                     kind="ExternalOutput")
    with tile.TileContext(nc, linearize=bool(int(os.environ.get(
            "KERNEL_LINEARIZE", "0")))) as tc:
        _emit(nc, tc, tin, t_out, dbg)
    nc.compile()
    return nc


def _emit(nc, tc, tin, t_out, dbg):
    import contextlib
    ctx = contextlib.ExitStack()
    AX = mybir.AxisListType.X
    OP = mybir.AluOpType
    ACT = mybir.ActivationFunctionType

    const = ctx.enter_context(tc.tile_pool(name="const", bufs=1))
    rows = ctx.enter_context(tc.tile_pool(name="rows", bufs=1))
    work = ctx.enter_context(tc.tile_pool(name="work", bufs=3))
    scr = ctx.enter_context(tc.tile_pool(name="scr", bufs=3))
    ps_bc = ctx.enter_context(tc.tile_pool(name="psbc", bufs=2, space="PSUM"))
    ps_mm = ctx.enter_context(tc.tile_pool(name="psmm", bufs=4, space="PSUM"))
    ps_st = ctx.enter_context(tc.tile_pool(name="psst", bufs=1, space="PSUM"))
    ps_sm = ctx.enter_context(tc.tile_pool(name="pssm", bufs=1, space="PSUM"))

    def bigtile(pool, tag="mmw"):
        bt = pool.tile([128, 512], F32, tag=tag, name="bt")
        return bt

    wb = const.tile([128, WF_TOT], BF16, tag="wb")
    zeros512 = const.tile([128, 512], BF16, tag="zeros512")
    bp = const.tile([128, BF_TOT], F32, tag="bp")
    xTb = const.tile([128, NCG * N], BF16, tag="xTb")  # x -> xwf -> hp
    Call = const.tile([128, NCG * 2048], BF16, tag="Call")
    xcatT = const.tile([128, NCG * 1536], BF16, tag="xcatT")
    msqcolf = const.tile([128, 64], F32, tag="msqcolf")
    gqcol = const.tile([128, 64], F32, tag="gqcol")

    def W(nm):
        o, w = WOFF[nm]
        return wb[:, o:o + w]

    def Bc(nm):
        o, w = BOFF[nm]
        return bp[:, o:o + w]

    def csel(cg):
        o, _ = WOFF["csel"]
        return wb[:, o + cg * 16: o + (cg + 1) * 16]

    def rself(cg):
        o, _ = WOFF["rsel"]
        return wb[0:16, o + cg * 128: o + (cg + 1) * 128]

    identf = Bc("identf")

    def bcast_row(row_tile, cg, n):
        pb = ps_bc.tile([128, 512], F32, tag="bcast")
        nc.tensor.matmul(pb[:, :n], lhsT=rself(cg), rhs=row_tile[0:16, 0:n],
                         start=True, stop=True)
        return pb

    def tcol(dst_col4, row_tile, pool=rows):
        """Transpose a [16,512] f32 row into 4 [128,16] column groups."""
        for sblk in range(4):
            pt = ps_bc.tile([128, 512], F32, tag="bcast")
            nc.tensor.transpose(pt[:, 0:16],
                                row_tile[:, sblk * 128:(sblk + 1) * 128],
                                identf[0:16, 0:16])
            nc.vector.tensor_copy(dst_col4[:, sblk * 16:(sblk + 1) * 16],
                                  pt[:, 0:16])

    nc.vector.memset(zeros512[:], 0.0)

    # ---- input DMAs (small first, then per-cg C + x chunks) ---------------
    nc.sync.dma_start(bp[:], tin["bpack"].ap())
    degr = rows.tile([16, N], F32, tag="degr")
    nc.scalar.dma_start(degr[:], tin["degr"].ap())
    nc.gpsimd.dma_start(wb[:, 0:384], tin["wpack"].ap()[:, 0:384])
    nc.gpsimd.dma_start(xTb[:, 0:4 * N], tin["xT"].ap()[:, 0:4 * N])
    for cg in range(4):
        nc.gpsimd.dma_start(Call[:, cg * 2048:(cg + 1) * 2048],
                            tin["cd"].ap()[:, cg * 2048:(cg + 1) * 2048])
    for h in range(1, 4):
        c0, c1 = h * 4, h * 4 + 4
        nc.gpsimd.dma_start(xTb[:, c0 * N:c1 * N],
                            tin["xT"].ap()[:, c0 * N:c1 * N])
        nc.gpsimd.dma_start(Call[:, c0 * 2048:c1 * 2048],
                            tin["cd"].ap()[:, c0 * 2048:c1 * 2048])
    nc.gpsimd.dma_start(wb[:, 384:], tin["wpack"].ap()[:, 384:])

    if DEBUG:
        nc.sync.dma_start(dbg["deg"].ap(), degr[:])

    # ---- GCN wavefront ----------------------------------------------------
    items = [("gcn", l, cg) for l in range(3) for cg in range(NCG)]

    def key(it):
        kind, l, cg = it
        return 3.0 * cg + 0.5 + 8.25 * l

    items.sort(key=key)

    meanT = rows.tile([128, 48], F32, tag="meanT")

    def emit_apply(l, cg, xws):
        ph = ps_mm.tile([128, 512], F32, tag="mmw")
        for sblk in range(4):
            nc.tensor.matmul(
                ph[:],
                lhsT=xws[:, sblk * 128:(sblk + 1) * 128],
                rhs=Call[:, cg * 2048 + sblk * 512:
                         cg * 2048 + (sblk + 1) * 512],
                start=(sblk == 0), stop=(sblk == 3))
        xout = xcatT[:, cg * 1536 + l * 512: cg * 1536 + (l + 1) * 512]
        mcol = meanT[:, l * 16 + cg: l * 16 + cg + 1]
        if (l + cg) % 2 == 0:
            nc.scalar.activation(xout, ph[:], ACT.Relu,
                                 bias=Bc("bcols")[:, l:l + 1],
                                 accum_out=mcol)
        else:
            nc.vector.scalar_tensor_tensor(
                xout, ph[:], Bc("bcols")[:, l:l + 1], zeros512[:],
                op0=OP.add, op1=OP.max, accum_out=mcol)

    pending = None
    for kind, l, cg in items:
        if l < 3:
            wl = W(("W1", "W2", "W3")[l])
            xws = work.tile([128, 512], BF16, tag="xws")
            pxw = ps_mm.tile([128, 512], F32, tag="mmw")
            for nt in range(4):
                if l == 0:
                    lhsT = xTb[:, cg * N + nt * 128: cg * N + (nt + 1) * 128]
                else:
                    lhsT = xcatT[:, cg * 1536 + (l - 1) * 512 + nt * 128:
                                 cg * 1536 + (l - 1) * 512 + (nt + 1) * 128]
                nc.tensor.matmul(pxw[:, nt * 128:(nt + 1) * 128], lhsT=lhsT,
                                 rhs=wl, start=True, stop=True)
            nc.vector.tensor_copy(xws[:], pxw[:])
            # software pipeline: emit the A-apply of the PREVIOUS item so
            # the PE never waits on this item's PSUM->SBUF copy
            if pending is not None:
                emit_apply(*pending)
            pending = (l, cg, xws)
    if pending is not None:
        emit_apply(*pending)

    def emit_xwf(cg):
        # XWf = xcat @ Wf for all nodes (pre-mask), node-major
        pxp = ps_mm.tile([128, 512], F32, tag="mmw", name="pxp")
        for nt in range(4):
            for ci in range(3):
                nc.tensor.matmul(
                    pxp[:, nt * 128:(nt + 1) * 128],
                    lhsT=xcatT[:, cg * 1536 + ci * 512 + nt * 128:
                               cg * 1536 + ci * 512 + (nt + 1) * 128],
                    rhs=W(f"Wf{ci}"), start=(ci == 0), stop=(ci == 2))
        nc.scalar.activation(xTb[:, cg * N:(cg + 1) * N], pxp[:], ACT.Copy)
    xwf = xTb
    if DEBUG:
        nc.sync.dma_start(dbg["xcatT"].ap(), xcatT[:])
        nc.sync.dma_start(dbg["C"].ap(), Call[:])

    if STAGE < 2:
        o3 = rows.tile([2, NPC], F32, tag="o3")
        nc.vector.memset(o3[:], 0.0)
        nc.sync.dma_start(t_out.ap(), o3[:])
        ctx.close()
        return

    # ---- attention pool (cT -> alpha -> gp); mean accumulated in-layer ----
    meanTb = rows.tile([128, 48], BF16, tag="meanTb")
    nc.scalar.activation(meanTb[:], meanT[:], ACT.Copy, scale=1.0 / N)
    if DEBUG:
        nc.sync.dma_start(dbg["meanT"].ap(), meanT[:])

    for cg in range(4):
        emit_xwf(cg)
    cT = rows.tile([128, 48], F32, tag="cT")
    for fo in range(3):
        pc = ps_sm.tile([128, 16], F32, tag="s16")
        for fi in range(3):
            nc.tensor.matmul(pc[:],
                             lhsT=W(f"Wg{fi}")[:, fo * 128:(fo + 1) * 128],
                             rhs=meanTb[:, fi * 16:(fi + 1) * 16],
                             start=(fi == 0), stop=(fi == 2))
        nc.scalar.activation(cT[:, fo * 16:(fo + 1) * 16], pc[:], ACT.Tanh)

    ps_al = ps_st.tile([16, N], F32, tag="stat")
    alq = []
    for cg in range(NCG):
        for ch in range(3):
            mlh = work.tile([128, 16], BF16, tag="mlh")
            nc.scalar.activation(mlh[:], csel(cg), ACT.Copy,
                                 scale=cT[:, ch * 16 + cg: ch * 16 + cg + 1])
            alq.append((mlh, cg, ch))
            if len(alq) > 1:
                m0, c0, h0 = alq.pop(0)
                nc.tensor.matmul(
                    ps_al[:], lhsT=m0[:],
                    rhs=xcatT[:, c0 * 1536 + h0 * 512:
                              c0 * 1536 + (h0 + 1) * 512],
                    start=(c0 == 0 and h0 == 0), stop=False)
    m0, c0, h0 = alq.pop(0)
    nc.tensor.matmul(
        ps_al[:], lhsT=m0[:],
        rhs=xcatT[:, c0 * 1536 + h0 * 512: c0 * 1536 + (h0 + 1) * 512],
        start=False, stop=True)
    alpha_row = rows.tile([16, N], BF16, tag="alpha")
    nc.scalar.activation(alpha_row[:], ps_al[:], ACT.Sigmoid)
    for cg in range(4, 6):
        emit_xwf(cg)
    if DEBUG:
        alpha_f = rows.tile([16, N], F32, tag="alphaf")
        nc.vector.tensor_copy(alpha_f[:], alpha_row[:])
        nc.sync.dma_start(dbg["alpha"].ap(), alpha_f[:])

    gpT = rows.tile([128, 48], F32, tag="gpT")
    for cg in range(NCG):
        pab = bcast_row(alpha_row, cg, N)
        for ch in range(3):
            sc = scr.tile([128, 512], BF16, tag="scr")
            nc.vector.scalar_tensor_tensor(
                sc[:], xcatT[:, cg * 1536 + ch * 512: cg * 1536 + (ch + 1) * 512],
                1.0, pab[:], op0=OP.mult, op1=OP.mult,
                accum_out=gpT[:, ch * 16 + cg: ch * 16 + cg + 1])
    if DEBUG:
        nc.sync.dma_start(dbg["gpT"].ap(), gpT[:])

    # ---- att_lin: pv = [gp1, gp2] @ Wal + bal -----------------------------
    gpcatTb = rows.tile([128, 48], BF16, tag="gpcatTb")
    for j in range(6):
        comp, ch = j // 3, j % 3
        nc.vector.tensor_copy(
            gpcatTb[:, j * 8:(j + 1) * 8],
            gpT[:, ch * 16 + comp * 8: ch * 16 + comp * 8 + 8])
    pvTb = rows.tile([128, 48], BF16, tag="pvTb")
    pvTf = rows.tile([128, 48], F32, tag="pvTf")
    for co in range(6):
        pp = ps_sm.tile([128, 16], F32, tag="s16")
        for ci in range(6):
            nc.tensor.matmul(pp[:, 0:8],
                             lhsT=W(f"Wal{ci}")[:, co * 128:(co + 1) * 128],
                             rhs=gpcatTb[:, ci * 8:(ci + 1) * 8],
                             start=(ci == 0), stop=(ci == 5))
        nc.vector.tensor_scalar(pvTf[:, co * 8:(co + 1) * 8], pp[:, 0:8],
                                Bc("balcol")[:, co:co + 1], None, op0=OP.add)
        nc.vector.tensor_copy(pvTb[:, co * 8:(co + 1) * 8],
                              pvTf[:, co * 8:(co + 1) * 8])
    if DEBUG:
        nc.sync.dma_start(dbg["pvT"].ap(), pvTf[:])

    # ---- ||pv|| then scores ----------------------------------------------
    rsncol = rows.tile([16, 1], F32, tag="rsncol")
    pn = ps_sm.tile([128, 16], F32, tag="s16")
    for ci in range(6):
        comp = ci // 3
        mpv = work.tile([128, 16], BF16, tag="mlh")
        nc.vector.memset(mpv[:], 0.0)
        nc.vector.tensor_copy(mpv[:, comp * 8:(comp + 1) * 8],
                              pvTb[:, ci * 8:(ci + 1) * 8])
        nc.tensor.matmul(pn[0:16, :], lhsT=mpv[:], rhs=mpv[:],
                         start=(ci == 0), stop=(ci == 5))
    dd = rows.tile([16, 16], F32, tag="dd")
    nc.vector.tensor_tensor(dd[:], pn[0:16, :], identf[0:16, 0:16],
                            op=OP.mult)
    nn = rows.tile([16, 1], F32, tag="nn")
    nc.vector.tensor_reduce(nn[:], dd[:], axis=AX, op=OP.add)
    sqn = rows.tile([16, 1], F32, tag="sqn")
    nc.scalar.activation(sqn[:], nn[:], ACT.Sqrt)
    nc.vector.reciprocal_approx_fast(rsncol[:], sqn[:])

    ps_sc = ps_st.tile([16, N], F32, tag="stat")
    scq = []
    for cg in range(NCG):
        comp, g = cg // NPC, cg % NPC
        for ci in range(3):
            mlh = work.tile([128, 16], BF16, tag="mlh")
            nc.scalar.activation(
                mlh[:], csel(cg), ACT.Copy,
                scale=pvTf[:, (comp * 3 + ci) * 8 + g:
                           (comp * 3 + ci) * 8 + g + 1])
            scq.append((mlh, cg, ci))
            if len(scq) > 1:
                m0, c0, h0 = scq.pop(0)
                nc.tensor.matmul(
                    ps_sc[:], lhsT=m0[:],
                    rhs=xcatT[:, c0 * 1536 + h0 * 512:
                              c0 * 1536 + (h0 + 1) * 512],
                    start=(c0 == 0 and h0 == 0), stop=False)
    m0, c0, h0 = scq.pop(0)
    nc.tensor.matmul(
        ps_sc[:], lhsT=m0[:],
        rhs=xcatT[:, c0 * 1536 + h0 * 512: c0 * 1536 + (h0 + 1) * 512],
        start=False, stop=True)
    score_row = rows.tile([16, N], F32, tag="score")
    nc.scalar.activation(score_row[:], ps_sc[:], ACT.Copy, scale=rsncol[:])
    for cg in range(6, NCG):
        emit_xwf(cg)
    if DEBUG:
        nc.sync.dma_start(dbg["scores"].ap(), score_row[:])

    if STAGE < 3:
        o3 = rows.tile([2, NPC], F32, tag="o3")
        nc.vector.memset(o3[:], 0.0)
        nc.sync.dma_start(t_out.ap(), o3[:])
        ctx.close()
        return

    # ---- top-256 mask (32 rounds of max8 + match_replace) -----------------
    cur = rows.tile([16, N], F32, tag="cur")
    nc.vector.tensor_copy(cur[:], score_row[:])
    mx = rows.tile([16, 8], F32, tag="mx")
    for _ in range(K1 // 8):
        nc.vector.max(out=mx[:], in_=cur[:])
        nc.vector.match_replace(out=cur[:], in_to_replace=mx[:],
                                in_values=cur[:], imm_value=-1e30)
    mask_row = rows.tile([16, N], F32, tag="mask")
    nc.vector.tensor_tensor(mask_row[:], score_row[:], cur[:], op=OP.not_equal)
    if DEBUG:
        nc.sync.dma_start(dbg["mask"].ap(), mask_row[:])
    sig_row = rows.tile([16, N], F32, tag="sig")
    nc.scalar.activation(sig_row[:], score_row[:], ACT.Sigmoid)

    sq_row = rows.tile([16, N], F32, tag="sq")
    nc.scalar.activation(sq_row[:], degr[:], ACT.Sqrt)
    msq_row = rows.tile([16, N], F32, tag="msq")
    nc.vector.tensor_tensor(msq_row[:], mask_row[:], sq_row[:], op=OP.mult)
    tcol(msqcolf, msq_row)

    # ---- pooled degree ----------------------------------------------------
    if STAGE < 4:
        o3 = rows.tile([2, NPC], F32, tag="o3")
        nc.vector.memset(o3[:], 0.0)
        nc.sync.dma_start(t_out.ap(), o3[:])
        ctx.close()
        return
    ps_d2 = ps_st.tile([16, N], F32, tag="stat")
    d2q = []
    for cg in range(NCG):
        for sblk in range(4):
            mlh = work.tile([128, 16], BF16, tag="mlh")
            mcol = msqcolf[:, sblk * 16 + cg: sblk * 16 + cg + 1]
            if sblk % 2 == 0:
                nc.scalar.activation(mlh[:], csel(cg), ACT.Copy, scale=mcol)
            else:
                nc.vector.tensor_scalar(mlh[:], csel(cg), mcol, None,
                                        op0=OP.mult)
            d2q.append((mlh, cg, sblk))
            if len(d2q) > 1:
                m0, c0, s0 = d2q.pop(0)
                nc.tensor.matmul(
                    ps_d2[:], lhsT=m0[:],
                    rhs=Call[:, c0 * 2048 + s0 * 512:
                             c0 * 2048 + (s0 + 1) * 512],
                    start=(c0 == 0 and s0 == 0), stop=False)
    m0, c0, s0 = d2q.pop(0)
    nc.tensor.matmul(
        ps_d2[:], lhsT=m0[:],
        rhs=Call[:, c0 * 2048 + s0 * 512: c0 * 2048 + (s0 + 1) * 512],
        start=False, stop=True)
    deg2_row = rows.tile([16, N], F32, tag="deg2")
    nc.vector.tensor_tensor(deg2_row[:], ps_d2[:], msq_row[:], op=OP.mult)
    nc.vector.tensor_tensor(deg2_row[:], deg2_row[:], mask_row[:],
                            op=OP.subtract)
    nc.vector.tensor_scalar(deg2_row[:], deg2_row[:], 1.0, None, op0=OP.add)
    sq2_row = rows.tile([16, N], F32, tag="sq2")
    nc.scalar.activation(sq2_row[:], deg2_row[:], ACT.Sqrt)
    rsd2_row = rows.tile([16, N], F32, tag="rsd2")
    nc.vector.reciprocal_approx_fast(rsd2_row[:], sq2_row[:])
    q_row = rows.tile([16, N], F32, tag="qrow")
    nc.vector.tensor_tensor(q_row[:], rsd2_row[:], msq_row[:], op=OP.mult)
    q_rowb = rows.tile([16, N], BF16, tag="qrowb")
    nc.vector.tensor_copy(q_rowb[:], q_row[:])
    gq_row = rows.tile([16, N], F32, tag="gqrow")
    nc.vector.scalar_tensor_tensor(gq_row[:], sig_row[:], 1.0, q_row[:],
                                   op0=OP.mult, op1=OP.mult)
    tcol(gqcol, gq_row)

    # ---- pooled conv (feat-major) + corrected mean pool -------------------
    # z[f,d] = sum_s C[s,d] gq_s xwf[s,f]; hp = relu(q_d z + bf).
    # Dropped dst cols have q_d = 0 so hp = relu(bf) there; the mean is
    # corrected by subtracting exactly (N-K1) relu(bf) per row, and the
    # final attention weights are masked, so those columns never leak.
    rbf256 = rows.tile([128, 1], F32, tag="rbf256")
    nc.scalar.activation(rbf256[:], Bc("bfcol"), ACT.Relu, scale=float(N - K1))
    rawsum = rows.tile([128, 16], F32, tag="rawsum")

    def emit_xwps(cg):
        xwps = work.tile([128, 512], BF16, tag="xws", name="xwps")
        for nt in range(4):
            sl_in = xwf[:, cg * N + nt * 128: cg * N + (nt + 1) * 128]
            sl_out = xwps[:, nt * 128:(nt + 1) * 128]
            gcol = gqcol[:, nt * 16 + cg: nt * 16 + cg + 1]
            nc.vector.tensor_scalar(sl_out, sl_in, gcol, None,
                                    op0=OP.mult)
        return xwps

    xwps_q = [emit_xwps(0)]
    for cg in range(NCG):
        if cg + 1 < NCG:
            xwps_q.append(emit_xwps(cg + 1))
        xwps = xwps_q.pop(0)
        z = ps_mm.tile([128, 512], F32, tag="mmw")
        for sblk in range(4):
            nc.tensor.matmul(
                z[:],
                lhsT=xwps[:, sblk * 128:(sblk + 1) * 128],
                rhs=Call[:, cg * 2048 + sblk * 512:
                         cg * 2048 + (sblk + 1) * 512],
                start=(sblk == 0), stop=(sblk == 3))
        bq = bcast_row(q_rowb, cg, N)
        bqs = scr.tile([128, 512], BF16, tag="scr")
        if cg % 2 == 0:
            nc.scalar.activation(bqs[:], bq[:], ACT.Copy)
        else:
            nc.vector.tensor_copy(bqs[:], bq[:])
        nc.vector.tensor_tensor(z[:], z[:], bqs[:], op=OP.mult)
        hp = xwf[:, cg * N:(cg + 1) * N]
        nc.scalar.activation(hp, z[:], ACT.Relu, bias=Bc("bfcol")[:, 0:1],
                             accum_out=rawsum[:, cg:cg + 1])
    hpall = xwf
    if DEBUG:
        nc.sync.dma_start(dbg["hp"].ap(), hpall[:])

    # ---- final attention pool (feat-major) --------------------------------
    mT2b = rows.tile([128, 16], BF16, tag="mT2b")
    nc.vector.tensor_scalar(mT2b[:], rawsum[:], rbf256[:, 0:1], 1.0 / K1,
                            op0=OP.subtract, op1=OP.mult)
    pc2 = ps_sm.tile([128, 16], F32, tag="s16")
    nc.tensor.matmul(pc2[:], lhsT=W("Wgf"), rhs=mT2b[:], start=True,
                     stop=True)
    c2Tf = rows.tile([128, 16], F32, tag="c2Tf")
    nc.scalar.activation(c2Tf[:], pc2[:], ACT.Tanh)

    ps_a2 = ps_st.tile([16, N], F32, tag="stat")
    a2q = []
    for cg in range(NCG):
        mlh = work.tile([128, 16], BF16, tag="mlh")
        nc.scalar.activation(mlh[:], csel(cg), ACT.Copy,
                             scale=c2Tf[:, cg:cg + 1])
        a2q.append((mlh, cg))
        if len(a2q) > 1:
            m0, c0 = a2q.pop(0)
            nc.tensor.matmul(ps_a2[:], lhsT=m0[:],
                             rhs=hpall[:, c0 * N:(c0 + 1) * N],
                             start=(c0 == 0), stop=False)
    m0, c0 = a2q.pop(0)
    nc.tensor.matmul(ps_a2[:], lhsT=m0[:], rhs=hpall[:, c0 * N:(c0 + 1) * N],
                     start=False, stop=True)
    wsum_row = rows.tile([16, N], F32, tag="wsum")
    nc.scalar.activation(wsum_row[:], ps_a2[:], ACT.Sigmoid)
    wsum_rowb = rows.tile([16, N], BF16, tag="wsumb")
    nc.vector.tensor_tensor(wsum_rowb[:], wsum_row[:], mask_row[:],
                            op=OP.mult)

    gcat = rows.tile([128, 16], F32, tag="gcat")
    for cg in range(NCG):
        bw = bcast_row(wsum_rowb, cg, N)
        sc3 = scr.tile([128, 512], BF16, tag="scr")
        nc.vector.scalar_tensor_tensor(
            sc3[:], hpall[:, cg * N:(cg + 1) * N], 1.0, bw[:],
            op0=OP.mult, op1=OP.mult, accum_out=gcat[:, cg:cg + 1])

    # ---- final MLP --------------------------------------------------------
    pcat = rows.tile([128, 16], BF16, tag="pcat")
    nc.vector.tensor_copy(pcat[:], gcat[:])
    p1b = bigtile(ps_mm)
    p1 = p1b[:, 0:128]
    nc.tensor.matmul(p1[:, 0:NPC], lhsT=W("Wl1a"), rhs=pcat[:, 0:NPC],
                     start=True, stop=False)
    nc.tensor.matmul(p1[:, 0:NPC], lhsT=W("Wl1b"), rhs=pcat[:, NPC:2 * NPC],
                     start=False, stop=True)
    o1 = rows.tile([128, NPC], BF16, tag="o1")
    nc.scalar.activation(o1[:], p1[:, 0:NPC], ACT.Relu, bias=Bc("bl1col")[:])
    p2b = bigtile(ps_mm)
    p2 = p2b[:, 0:128]
    nc.tensor.matmul(p2[0:64, 0:NPC], lhsT=W("Wl2"), rhs=o1[:], start=True,
                     stop=True)
    o2 = rows.tile([64, NPC], BF16, tag="o2")
    nc.scalar.activation(o2[:], p2[0:64, 0:NPC], ACT.Relu,
                         bias=Bc("bl2col")[0:64, :])
    p3b = bigtile(ps_mm)
    p3 = p3b[:, 0:128]
    nc.tensor.matmul(p3[0:2, 0:NPC], lhsT=W("Wl3")[0:64, :], rhs=o2[:],
                     start=True, stop=True)
    o3 = rows.tile([2, NPC], F32, tag="o3")
    nc.vector.tensor_scalar(o3[:], p3[0:2, 0:NPC], Bc("bl3col")[0:2, :],
                            None, op0=OP.add)
    nc.sync.dma_start(t_out.ap(), o3[:])
    ctx.close()


_NC_CACHE = {}


def _get_nc():
    key = (STAGE, DEBUG)
    if key not in _NC_CACHE:
        _NC_CACHE[key] = _build()
    return _NC_CACHE[key]


def kernel(**inputs):
    in_maps = _host_prep(inputs)
    nc = _get_nc()
    trace = bool(int(os.environ.get("KERNEL_TRACE", "0")))
    tmpdir = os.environ.get("KERNEL_TRACE_DIR") or None
    res = run_bass_kernel_spmd(nc, in_maps, core_ids=list(range(NCORES)),
                               trace=trace, tmpdir=tmpdir)
    out = np.empty((B, 2), np.float32)
    for c in range(NCORES):
        out[c * NPC:(c + 1) * NPC] = res.results[c]["out"].T
    kernel._last = res
    return out


# revision 30
# speedup vs baseline: 1.1914x; 1.0178x over previous
"""Trainium2 Bass kernel for nn_CAGpool (GNN message passing, CAG pooling).

Sharding: data-parallel over the 64 graph pairs -> 8 pairs (16 component
graphs of 512 nodes) per NeuronCore.  Message passing is dense matmul
against a per-graph 512x512 normalized adjacency (A+I with symmetric GCN
norm, a pure function of the edge-index structure) laid out host-side;
all compute on x/weights (GCN layers, attention pooling, top-k, pooled
conv, MLP) runs on device.

Per-core schedule: C/x stream in per-graph over SWDGE; the 3 GCN layers
run as a software-pipelined PE wavefront (the A-apply of item k-1 is
emitted under item k's PSUM->SBUF evacuation so the PE never stalls and
holds its fast p-state); the per-layer ReLU writebacks accumulate the
attention-pool means for free via activation accum_out.  Attention
scoring uses csel-selector matmuls whose selector builds sit on the
Scalar engine one step ahead of the PE.  The pooled-conv weight
precompute XWf = xcat @ Wf is mask-independent and is scheduled into the
otherwise PE-idle top-k window (top-k = 32 rounds of max8/match_replace
on Vector).  The pooled conv itself runs feat-major (4x 512-col matmuls
per graph); dropped nodes carry a zero dst-scale so their columns hold
relu(bf) exactly, which the mean pool corrects in closed form, and the
final attention weights are masked, so no per-column masking pass is
needed.
"""

import os
import numpy as np
import ml_dtypes

import concourse.bass as bass
import concourse.tile as tile
from concourse import bacc, mybir
from concourse.bass_utils import run_bass_kernel_spmd

F32 = mybir.dt.float32
BF16 = mybir.dt.bfloat16
F16 = mybir.dt.float16

NCORES = 8
B = 64
NPC = B // NCORES          # graph pairs per core (8)
NCG = 2 * NPC              # component graphs per core (16)
N = 512                    # nodes per component graph
K1 = 256
DEBUG = bool(int(os.environ.get("KERNEL_DEBUG", "0")))
STAGE = int(os.environ.get("KERNEL_STAGE", "4"))


def _layout(ent):
    offs, off = {}, 0
    for nm, w in ent:
        offs[nm] = (off, w)
        off += w
    return offs, off


WOFF, WF_TOT = _layout(
    [("W1", 128), ("W2", 128), ("W3", 128), ("Wgf", 128)]
    + [(f"Wg{i}", 384) for i in range(3)]
    + [(f"Wal{i}", 768) for i in range(6)]
    + [(f"Wf{i}", 128) for i in range(3)]
    + [("Wl1a", 128), ("Wl1b", 128), ("Wl2", 64), ("Wl3", 2),
       ("csel", 256), ("rsel", 2048)])
BOFF, BF_TOT = _layout(
    [("balcol", 6), ("bl1col", 1), ("bl2col", 1),
     ("bl3col", 1), ("identf", 128), ("bcols", 3), ("bfcol", 1)])


def _host_prep(inputs):
    """Per-core input maps. Integer index/count prep + dtype staging only."""
    x = np.asarray(inputs["x"], np.float32)

    s_loc, d_loc = {}, {}
    for comp, (sk, dk) in enumerate((("src_c1", "dst_c1"),
                                     ("src_c2", "dst_c2"))):
        base = (np.arange(B) * N)[:, None]
        s_loc[comp] = np.asarray(inputs[sk]).reshape(B, -1) - base
        d_loc[comp] = np.asarray(inputs[dk]).reshape(B, -1) - base

    in_maps = []
    for c in range(NCORES):
        xT = np.empty((128, NCG * N), ml_dtypes.bfloat16)
        cd = np.zeros((128, NCG * 2048), ml_dtypes.bfloat16)
        degr = np.empty((NCG, N), np.float32)
        for comp in range(2):
            for gl in range(NPC):
                g = c * NPC + gl
                cg = comp * NPC + gl
                r0 = g * 2 * N + comp * N
                xT[:, cg * N:(cg + 1) * N] = x[r0:r0 + N].T
                s = s_loc[comp][g].astype(np.int64)
                d = d_loc[comp][g].astype(np.int64)
                cnt = np.bincount(s * N + d, minlength=N * N)
                cmat = (cnt.reshape(N, N) + np.eye(N, dtype=np.int64)
                        ).astype(np.float32)
                dg = (np.bincount(d, minlength=N) + 1).astype(np.float32)
                degr[cg] = dg
                # symmetric gcn norm (graph-structure preprocessing)
                rsd = 1.0 / np.sqrt(dg)
                cmat *= rsd[:, None]
                cmat *= rsd[None, :]
                # [src, dst] -> [p=src%128, sblk*512 + dst]
                cd[:, cg * 2048:(cg + 1) * 2048] = (
                    cmat.reshape(4, 128, N).transpose(1, 0, 2)
                    .reshape(128, 2048))

        wpack = np.zeros((128, WF_TOT), np.float32)

        def put(nm, arr):
            o, w = WOFF[nm]
            arr = np.asarray(arr, np.float32)
            wpack[: arr.shape[0], o:o + arr.shape[1]] = arr

        put("W1", inputs["W1"]); put("W2", inputs["W2"]); put("W3", inputs["W3"])
        put("Wgf", inputs["Wg_fin"])
        for i in range(3):
            put(f"Wg{i}", np.asarray(inputs["Wg_att"])[i * 128:(i + 1) * 128])
        for i in range(6):
            put(f"Wal{i}", np.asarray(inputs["Wal"])[i * 128:(i + 1) * 128])
        for i in range(3):
            put(f"Wf{i}", np.asarray(inputs["Wf"])[i * 128:(i + 1) * 128])
        put("Wl1a", np.asarray(inputs["Wl1"])[:128])
        put("Wl1b", np.asarray(inputs["Wl1"])[128:])
        put("Wl2", inputs["Wl2"])
        put("Wl3", inputs["Wl3"])
        csel = np.zeros((128, 256), np.float32)
        for cg in range(NCG):
            csel[:, cg * 16 + cg] = 1.0
        put("csel", csel)
        rsel = np.zeros((16, 2048), np.float32)
        for cg in range(16):
            rsel[cg, cg * 128:(cg + 1) * 128] = 1.0
        put("rsel", rsel)

        bpack = np.zeros((128, BF_TOT), np.float32)

        def putb(nm, arr):
            o, w = BOFF[nm]
            arr = np.asarray(arr, np.float32)
            bpack[: arr.shape[0], o:o + arr.shape[1]] = arr

        putb("balcol", np.asarray(inputs["bal"]).reshape(6, 128).T)
        putb("bl1col", np.asarray(inputs["bl1"])[:, None])
        putb("bl2col", np.asarray(inputs["bl2"])[:, None])
        putb("bl3col", np.asarray(inputs["bl3"])[:, None])
        putb("bcols", np.stack([np.asarray(inputs["b1"]),
                                np.asarray(inputs["b2"]),
                                np.asarray(inputs["b3"])], 1))
        putb("identf", np.eye(128, dtype=np.float32))
        putb("bfcol", np.asarray(inputs["bf"])[:, None])

        in_maps.append({"xT": np.ascontiguousarray(xT),
                        "cd": np.ascontiguousarray(cd),
                        "degr": degr,
                        "wpack": wpack.astype(ml_dtypes.bfloat16),
                        "bpack": bpack})
    return in_maps


def _build():
    nc = bacc.Bacc("TRN2", target_bir_lowering=False, debug=False,
                   num_devices=NCORES)
    tin = {
        "xT": nc.dram_tensor("xT", [128, NCG * N], BF16, kind="ExternalInput"),
        "cd": nc.dram_tensor("cd", [128, NCG * 2048], BF16,
                             kind="ExternalInput"),
        "degr": nc.dram_tensor("degr", [NCG, N], F32, kind="ExternalInput"),
        "wpack": nc.dram_tensor("wpack", [128, WF_TOT], BF16,
                                kind="ExternalInput"),
        "bpack": nc.dram_tensor("bpack", [128, BF_TOT], F32,
                                kind="ExternalInput"),
    }
    t_out = nc.dram_tensor("out", [2, NPC], F32, kind="ExternalOutput")
    dbg = {}
    if DEBUG:
        for nm, shape, dt in (
                ("C", [128, NCG * 2048], BF16), ("deg", [16, N], F32),
                ("xcatT", [128, NCG * 1536], BF16), ("pvT", [128, 48], F32),
                ("scores", [16, N], F32), ("mask", [16, N], F32),
                ("alpha", [16, N], F32), ("gpT", [128, 48], F32),
                ("meanT", [128, 48], F32), ("hp", [128, NCG * 512], BF16)):
            dbg[nm] = nc.dram_tensor("dbg_" + nm, shape, dt,
                                     kind="ExternalOutput")
    with tile.TileContext(nc, linearize=bool(int(os.environ.get(
            "KERNEL_LINEARIZE", "0")))) as tc:
        _emit(nc, tc, tin, t_out, dbg)
    nc.compile()
    return nc


def _emit(nc, tc, tin, t_out, dbg):
    import contextlib
    ctx = contextlib.ExitStack()
    AX = mybir.AxisListType.X
    OP = mybir.AluOpType
    ACT = mybir.ActivationFunctionType

    const = ctx.enter_context(tc.tile_pool(name="const", bufs=1))
    rows = ctx.enter_context(tc.tile_pool(name="rows", bufs=1))
    work = ctx.enter_context(tc.tile_pool(name="work", bufs=3))
    scr = ctx.enter_context(tc.tile_pool(name="scr", bufs=3))
    ps_bc = ctx.enter_context(tc.tile_pool(name="psbc", bufs=2, space="PSUM"))
    ps_mm = ctx.enter_context(tc.tile_pool(name="psmm", bufs=4, space="PSUM"))
    ps_st = ctx.enter_context(tc.tile_pool(name="psst", bufs=1, space="PSUM"))
    ps_sm = ctx.enter_context(tc.tile_pool(name="pssm", bufs=1, space="PSUM"))

    def bigtile(pool, tag="mmw"):
        bt = pool.tile([128, 512], F32, tag=tag, name="bt")
        return bt

    wb = const.tile([128, WF_TOT], BF16, tag="wb")
    zeros512 = const.tile([128, 512], BF16, tag="zeros512")
    bp = const.tile([128, BF_TOT], F32, tag="bp")
    xTb = const.tile([128, NCG * N], BF16, tag="xTb")  # x -> xwf -> hp
    Call = const.tile([128, NCG * 2048], BF16, tag="Call")
    xcatT = const.tile([128, NCG * 1536], BF16, tag="xcatT")
    msqcolf = const.tile([128, 64], F32, tag="msqcolf")
    gqcol = const.tile([128, 64], F32, tag="gqcol")

    def W(nm):
        o, w = WOFF[nm]
        return wb[:, o:o + w]

    def Bc(nm):
        o, w = BOFF[nm]
        return bp[:, o:o + w]

    def csel(cg):
        o, _ = WOFF["csel"]
        return wb[:, o + cg * 16: o + (cg + 1) * 16]

    def rself(cg):
        o, _ = WOFF["rsel"]
        return wb[0:16, o + cg * 128: o + (cg + 1) * 128]

    identf = Bc("identf")

    def bcast_row(row_tile, cg, n):
        pb = ps_bc.tile([128, 512], F32, tag="bcast")
        nc.tensor.matmul(pb[:, :n], lhsT=rself(cg), rhs=row_tile[0:16, 0:n],
                         start=True, stop=True)
        return pb

    def tcol(dst_col4, row_tile, pool=rows):
        """Transpose a [16,512] f32 row into 4 [128,16] column groups."""
        for sblk in range(4):
            pt = ps_bc.tile([128, 512], F32, tag="bcast")
            nc.tensor.transpose(pt[:, 0:16],
                                row_tile[:, sblk * 128:(sblk + 1) * 128],
                                identf[0:16, 0:16])
            nc.vector.tensor_copy(dst_col4[:, sblk * 16:(sblk + 1) * 16],
                                  pt[:, 0:16])

    nc.vector.memset(zeros512[:], 0.0)

    # ---- input DMAs (small first, then per-cg C + x chunks) ---------------
    nc.sync.dma_start(bp[:], tin["bpack"].ap())
    degr = rows.tile([16, N], F32, tag="degr")
    nc.scalar.dma_start(degr[:], tin["degr"].ap())
    nc.gpsimd.dma_start(wb[:, 0:384], tin["wpack"].ap()[:, 0:384])
    nc.gpsimd.dma_start(xTb[:, 0:4 * N], tin["xT"].ap()[:, 0:4 * N])
    for cg in range(4):
        nc.gpsimd.dma_start(Call[:, cg * 2048:(cg + 1) * 2048],
                            tin["cd"].ap()[:, cg * 2048:(cg + 1) * 2048])
    for h in range(1, 4):
        c0, c1 = h * 4, h * 4 + 4
        nc.gpsimd.dma_start(xTb[:, c0 * N:c1 * N],
                            tin["xT"].ap()[:, c0 * N:c1 * N])
        nc.gpsimd.dma_start(Call[:, c0 * 2048:c1 * 2048],
                            tin["cd"].ap()[:, c0 * 2048:c1 * 2048])
    nc.gpsimd.dma_start(wb[:, 384:], tin["wpack"].ap()[:, 384:])

    if DEBUG:
        nc.sync.dma_start(dbg["deg"].ap(), degr[:])

    # ---- GCN wavefront ----------------------------------------------------
    items = [("gcn", l, cg) for l in range(3) for cg in range(NCG)]

    def key(it):
        kind, l, cg = it
        return 3.0 * cg + 0.5 + 8.25 * l

    items.sort(key=key)

    meanT = rows.tile([128, 48], F32, tag="meanT")

    def emit_apply(l, cg, xws):
        ph = ps_mm.tile([128, 512], F32, tag="mmw")
        for sblk in range(4):
            nc.tensor.matmul(
                ph[:],
                lhsT=xws[:, sblk * 128:(sblk + 1) * 128],
                rhs=Call[:, cg * 2048 + sblk * 512:
                         cg * 2048 + (sblk + 1) * 512],
                start=(sblk == 0), stop=(sblk == 3))
        xout = xcatT[:, cg * 1536 + l * 512: cg * 1536 + (l + 1) * 512]
        mcol = meanT[:, l * 16 + cg: l * 16 + cg + 1]
        if (l + cg) % 2 == 0:
            nc.scalar.activation(xout, ph[:], ACT.Relu,
                                 bias=Bc("bcols")[:, l:l + 1],
                                 accum_out=mcol)
        else:
            nc.vector.scalar_tensor_tensor(
                xout, ph[:], Bc("bcols")[:, l:l + 1], zeros512[:],
                op0=OP.add, op1=OP.max, accum_out=mcol)

    pending = None
    for kind, l, cg in items:
        if l < 3:
            wl = W(("W1", "W2", "W3")[l])
            xws = work.tile([128, 512], BF16, tag="xws")
            pxw = ps_mm.tile([128, 512], F32, tag="mmw")
            for nt in range(4):
                if l == 0:
                    lhsT = xTb[:, cg * N + nt * 128: cg * N + (nt + 1) * 128]
                else:
                    lhsT = xcatT[:, cg * 1536 + (l - 1) * 512 + nt * 128:
                                 cg * 1536 + (l - 1) * 512 + (nt + 1) * 128]
                nc.tensor.matmul(pxw[:, nt * 128:(nt + 1) * 128], lhsT=lhsT,
                                 rhs=wl, start=True, stop=True)
            nc.vector.tensor_copy(xws[:], pxw[:])
            # software pipeline: emit the A-apply of the PREVIOUS item so
            # the PE never waits on this item's PSUM->SBUF copy
            if pending is not None:
                emit_apply(*pending)
            pending = (l, cg, xws)
    if pending is not None:
        emit_apply(*pending)

    def emit_xwf(cg):
        # XWf = xcat @ Wf for all nodes (pre-mask), node-major
        pxp = ps_mm.tile([128, 512], F32, tag="mmw", name="pxp")
        for nt in range(4):
            for ci in range(3):
                nc.tensor.matmul(
                    pxp[:, nt * 128:(nt + 1) * 128],
                    lhsT=xcatT[:, cg * 1536 + ci * 512 + nt * 128:
                               cg * 1536 + ci * 512 + (nt + 1) * 128],
                    rhs=W(f"Wf{ci}"), start=(ci == 0), stop=(ci == 2))
        nc.scalar.activation(xTb[:, cg * N:(cg + 1) * N], pxp[:], ACT.Copy)
    xwf = xTb
    if DEBUG:
        nc.sync.dma_start(dbg["xcatT"].ap(), xcatT[:])
        nc.sync.dma_start(dbg["C"].ap(), Call[:])

    if STAGE < 2:
        o3 = rows.tile([2, NPC], F32, tag="o3")
        nc.vector.memset(o3[:], 0.0)
        nc.sync.dma_start(t_out.ap(), o3[:])
        ctx.close()
        return

    # ---- attention pool (cT -> alpha -> gp); mean accumulated in-layer ----
    meanTb = rows.tile([128, 48], BF16, tag="meanTb")
    nc.scalar.activation(meanTb[:], meanT[:], ACT.Copy, scale=1.0 / N)
    if DEBUG:
        nc.sync.dma_start(dbg["meanT"].ap(), meanT[:])

    for cg in range(4):
        emit_xwf(cg)
    cT = rows.tile([128, 48], F32, tag="cT")
    for fo in range(3):
        pc = ps_sm.tile([128, 16], F32, tag="s16")
        for fi in range(3):
            nc.tensor.matmul(pc[:],
                             lhsT=W(f"Wg{fi}")[:, fo * 128:(fo + 1) * 128],
                             rhs=meanTb[:, fi * 16:(fi + 1) * 16],
                             start=(fi == 0), stop=(fi == 2))
        nc.scalar.activation(cT[:, fo * 16:(fo + 1) * 16], pc[:], ACT.Tanh)

    ps_al = ps_st.tile([16, N], F32, tag="stat")
    alq = []
    for cg in range(NCG):
        for ch in range(3):
            mlh = work.tile([128, 16], BF16, tag="mlh")
            nc.scalar.activation(mlh[:], csel(cg), ACT.Copy,
                                 scale=cT[:, ch * 16 + cg: ch * 16 + cg + 1])
            alq.append((mlh, cg, ch))
            if len(alq) > 1:
                m0, c0, h0 = alq.pop(0)
                nc.tensor.matmul(
                    ps_al[:], lhsT=m0[:],
                    rhs=xcatT[:, c0 * 1536 + h0 * 512:
                              c0 * 1536 + (h0 + 1) * 512],
                    start=(c0 == 0 and h0 == 0), stop=False)
    m0, c0, h0 = alq.pop(0)
    nc.tensor.matmul(
        ps_al[:], lhsT=m0[:],
        rhs=xcatT[:, c0 * 1536 + h0 * 512: c0 * 1536 + (h0 + 1) * 512],
        start=False, stop=True)
    alpha_row = rows.tile([16, N], BF16, tag="alpha")
    nc.scalar.activation(alpha_row[:], ps_al[:], ACT.Sigmoid)
    for cg in range(4, 6):
        emit_xwf(cg)
    if DEBUG:
        alpha_f = rows.tile([16, N], F32, tag="alphaf")
        nc.vector.tensor_copy(alpha_f[:], alpha_row[:])
        nc.sync.dma_start(dbg["alpha"].ap(), alpha_f[:])

    gpT = rows.tile([128, 48], F32, tag="gpT")
    for cg in range(NCG):
        pab = bcast_row(alpha_row, cg, N)
        for ch in range(3):
            sc = scr.tile([128, 512], BF16, tag="scr")
            nc.vector.scalar_tensor_tensor(
                sc[:], xcatT[:, cg * 1536 + ch * 512: cg * 1536 + (ch + 1) * 512],
                1.0, pab[:], op0=OP.mult, op1=OP.mult,
                accum_out=gpT[:, ch * 16 + cg: ch * 16 + cg + 1])
    if DEBUG:
        nc.sync.dma_start(dbg["gpT"].ap(), gpT[:])

    # ---- att_lin: pv = [gp1, gp2] @ Wal + bal -----------------------------
    gpcatTb = rows.tile([128, 48], BF16, tag="gpcatTb")
    for j in range(6):
        comp, ch = j // 3, j % 3
        nc.vector.tensor_copy(
            gpcatTb[:, j * 8:(j + 1) * 8],
            gpT[:, ch * 16 + comp * 8: ch * 16 + comp * 8 + 8])
    pvTb = rows.tile([128, 48], BF16, tag="pvTb")
    pvTf = rows.tile([128, 48], F32, tag="pvTf")
    for co in range(6):
        pp = ps_sm.tile([128, 16], F32, tag="s16")
        for ci in range(6):
            nc.tensor.matmul(pp[:, 0:8],
                             lhsT=W(f"Wal{ci}")[:, co * 128:(co + 1) * 128],
                             rhs=gpcatTb[:, ci * 8:(ci + 1) * 8],
                             start=(ci == 0), stop=(ci == 5))
        nc.vector.tensor_scalar(pvTf[:, co * 8:(co + 1) * 8], pp[:, 0:8],
                                Bc("balcol")[:, co:co + 1], None, op0=OP.add)
        nc.vector.tensor_copy(pvTb[:, co * 8:(co + 1) * 8],
                              pvTf[:, co * 8:(co + 1) * 8])
    if DEBUG:
        nc.sync.dma_start(dbg["pvT"].ap(), pvTf[:])

    # ---- ||pv|| then scores ----------------------------------------------
    rsncol = rows.tile([16, 1], F32, tag="rsncol")
    pn = ps_sm.tile([128, 16], F32, tag="s16")
    for ci in range(6):
        comp = ci // 3
        mpv = work.tile([128, 16], BF16, tag="mlh")
        nc.vector.memset(mpv[:], 0.0)
        nc.vector.tensor_copy(mpv[:, comp * 8:(comp + 1) * 8],
                              pvTb[:, ci * 8:(ci + 1) * 8])
        nc.tensor.matmul(pn[0:16, :], lhsT=mpv[:], rhs=mpv[:],
                         start=(ci == 0), stop=(ci == 5))
    dd = rows.tile([16, 16], F32, tag="dd")
    nc.vector.tensor_tensor(dd[:], pn[0:16, :], identf[0:16, 0:16],
                            op=OP.mult)
    nn = rows.tile([16, 1], F32, tag="nn")
    nc.vector.tensor_reduce(nn[:], dd[:], axis=AX, op=OP.add)
    sqn = rows.tile([16, 1], F32, tag="sqn")
    nc.scalar.activation(sqn[:], nn[:], ACT.Sqrt)
    nc.vector.reciprocal_approx_fast(rsncol[:], sqn[:])

    ps_sc = ps_st.tile([16, N], F32, tag="stat")
    scq = []
    for cg in range(NCG):
        comp, g = cg // NPC, cg % NPC
        for ci in range(3):
            mlh = work.tile([128, 16], BF16, tag="mlh")
            nc.scalar.activation(
                mlh[:], csel(cg), ACT.Copy,
                scale=pvTf[:, (comp * 3 + ci) * 8 + g:
                           (comp * 3 + ci) * 8 + g + 1])
            scq.append((mlh, cg, ci))
            if len(scq) > 1:
                m0, c0, h0 = scq.pop(0)
                nc.tensor.matmul(
                    ps_sc[:], lhsT=m0[:],
                    rhs=xcatT[:, c0 * 1536 + h0 * 512:
                              c0 * 1536 + (h0 + 1) * 512],
                    start=(c0 == 0 and h0 == 0), stop=False)
    m0, c0, h0 = scq.pop(0)
    nc.tensor.matmul(
        ps_sc[:], lhsT=m0[:],
        rhs=xcatT[:, c0 * 1536 + h0 * 512: c0 * 1536 + (h0 + 1) * 512],
        start=False, stop=True)
    score_row = rows.tile([16, N], F32, tag="score")
    nc.scalar.activation(score_row[:], ps_sc[:], ACT.Copy, scale=rsncol[:])
    score16 = rows.tile([16, N], F16, tag="score16")
    nc.vector.tensor_copy(score16[:], score_row[:])
    for cg in range(6, NCG):
        emit_xwf(cg)
    if DEBUG:
        nc.sync.dma_start(dbg["scores"].ap(), score_row[:])

    if STAGE < 3:
        o3 = rows.tile([2, NPC], F32, tag="o3")
        nc.vector.memset(o3[:], 0.0)
        nc.sync.dma_start(t_out.ap(), o3[:])
        ctx.close()
        return

    # ---- top-256 mask (32 rounds of fp16 max8 + match_replace) ------------
    # match_replace is positional, so fp16 ties still yield exactly K1 kept
    # positions; a tie can only swap nodes whose scores differ by < 1 fp16
    # ulp, which is well inside the accuracy budget.
    cur = rows.tile([16, N], F16, tag="cur")
    nc.vector.tensor_copy(cur[:], score16[:])
    mx = rows.tile([16, 8], F16, tag="mx")
    for _ in range(K1 // 8):
        nc.vector.max(out=mx[:], in_=cur[:])
        nc.vector.match_replace(out=cur[:], in_to_replace=mx[:],
                                in_values=cur[:], imm_value=-60000.0)
    mask_row = rows.tile([16, N], F32, tag="mask")
    nc.vector.tensor_tensor(mask_row[:], score16[:], cur[:], op=OP.not_equal)
    if DEBUG:
        nc.sync.dma_start(dbg["mask"].ap(), mask_row[:])
    sig_row = rows.tile([16, N], F32, tag="sig")
    nc.scalar.activation(sig_row[:], score_row[:], ACT.Sigmoid)

    sq_row = rows.tile([16, N], F32, tag="sq")
    nc.scalar.activation(sq_row[:], degr[:], ACT.Sqrt)
    msq_row = rows.tile([16, N], F32, tag="msq")
    nc.vector.tensor_tensor(msq_row[:], mask_row[:], sq_row[:], op=OP.mult)
    tcol(msqcolf, msq_row)

    # ---- pooled degree ----------------------------------------------------
    if STAGE < 4:
        o3 = rows.tile([2, NPC], F32, tag="o3")
        nc.vector.memset(o3[:], 0.0)
        nc.sync.dma_start(t_out.ap(), o3[:])
        ctx.close()
        return
    ps_d2 = ps_st.tile([16, N], F32, tag="stat")
    d2q = []
    for cg in range(NCG):
        for sblk in range(4):
            mlh = work.tile([128, 16], BF16, tag="mlh")
            mcol = msqcolf[:, sblk * 16 + cg: sblk * 16 + cg + 1]
            if sblk % 2 == 0:
                nc.scalar.activation(mlh[:], csel(cg), ACT.Copy, scale=mcol)
            else:
                nc.vector.tensor_scalar(mlh[:], csel(cg), mcol, None,
                                        op0=OP.mult)
            d2q.append((mlh, cg, sblk))
            if len(d2q) > 1:
                m0, c0, s0 = d2q.pop(0)
                nc.tensor.matmul(
                    ps_d2[:], lhsT=m0[:],
                    rhs=Call[:, c0 * 2048 + s0 * 512:
                             c0 * 2048 + (s0 + 1) * 512],
                    start=(c0 == 0 and s0 == 0), stop=False)
    m0, c0, s0 = d2q.pop(0)
    nc.tensor.matmul(
        ps_d2[:], lhsT=m0[:],
        rhs=Call[:, c0 * 2048 + s0 * 512: c0 * 2048 + (s0 + 1) * 512],
        start=False, stop=True)
    deg2_row = rows.tile([16, N], F32, tag="deg2")
    nc.vector.tensor_tensor(deg2_row[:], ps_d2[:], msq_row[:], op=OP.mult)
    nc.vector.tensor_tensor(deg2_row[:], deg2_row[:], mask_row[:],
                            op=OP.subtract)
    nc.vector.tensor_scalar(deg2_row[:], deg2_row[:], 1.0, None, op0=OP.add)
    sq2_row = rows.tile([16, N], F32, tag="sq2")
    nc.scalar.activation(sq2_row[:], deg2_row[:], ACT.Sqrt)
    rsd2_row = rows.tile([16, N], F32, tag="rsd2")
    nc.vector.reciprocal_approx_fast(rsd2_row[:], sq2_row[:])
    q_row = rows.tile([16, N], F32, tag="qrow")
    nc.vector.tensor_tensor(q_row[:], rsd2_row[:], msq_row[:], op=OP.mult)
    q_rowb = rows.tile([16, N], BF16, tag="qrowb")
    nc.vector.tensor_copy(q_rowb[:], q_row[:])
    gq_row = rows.tile([16, N], F32, tag="gqrow")
    nc.vector.scalar_tensor_tensor(gq_row[:], sig_row[:], 1.0, q_row[:],
                                   op0=OP.mult, op1=OP.mult)
    tcol(gqcol, gq_row)

    # ---- pooled conv (feat-major) + corrected mean pool -------------------
    # z[f,d] = sum_s C[s,d] gq_s xwf[s,f]; hp = relu(q_d z + bf).
    # Dropped dst cols have q_d = 0 so hp = relu(bf) there; the mean is
    # corrected by subtracting exactly (N-K1) relu(bf) per row, and the
    # final attention weights are masked, so those columns never leak.
    rbf256 = rows.tile([128, 1], F32, tag="rbf256")
    nc.scalar.activation(rbf256[:], Bc("bfcol"), ACT.Relu, scale=float(N - K1))
    rawsum = rows.tile([128, 16], F32, tag="rawsum")

    def emit_xwps(cg):
        xwps = work.tile([128, 512], BF16, tag="xws", name="xwps")
        for nt in range(4):
            sl_in = xwf[:, cg * N + nt * 128: cg * N + (nt + 1) * 128]
            sl_out = xwps[:, nt * 128:(nt + 1) * 128]
            gcol = gqcol[:, nt * 16 + cg: nt * 16 + cg + 1]
            nc.vector.tensor_scalar(sl_out, sl_in, gcol, None,
                                    op0=OP.mult)
        return xwps

    xwps_q = [emit_xwps(0)]
    for cg in range(NCG):
        if cg + 1 < NCG:
            xwps_q.append(emit_xwps(cg + 1))
        xwps = xwps_q.pop(0)
        z = ps_mm.tile([128, 512], F32, tag="mmw")
        for sblk in range(4):
            nc.tensor.matmul(
                z[:],
                lhsT=xwps[:, sblk * 128:(sblk + 1) * 128],
                rhs=Call[:, cg * 2048 + sblk * 512:
                         cg * 2048 + (sblk + 1) * 512],
                start=(sblk == 0), stop=(sblk == 3))
        bq = bcast_row(q_rowb, cg, N)
        bqs = scr.tile([128, 512], BF16, tag="scr")
        if cg % 2 == 0:
            nc.scalar.activation(bqs[:], bq[:], ACT.Copy)
        else:
            nc.vector.tensor_copy(bqs[:], bq[:])
        nc.vector.tensor_tensor(z[:], z[:], bqs[:], op=OP.mult)
        hp = xwf[:, cg * N:(cg + 1) * N]
        nc.scalar.activation(hp, z[:], ACT.Relu, bias=Bc("bfcol")[:, 0:1],
                             accum_out=rawsum[:, cg:cg + 1])
    hpall = xwf
    if DEBUG:
        nc.sync.dma_start(dbg["hp"].ap(), hpall[:])

    # ---- final attention pool (feat-major) --------------------------------
    mT2b = rows.tile([128, 16], BF16, tag="mT2b")
    nc.vector.tensor_scalar(mT2b[:], rawsum[:], rbf256[:, 0:1], 1.0 / K1,
                            op0=OP.subtract, op1=OP.mult)
    pc2 = ps_sm.tile([128, 16], F32, tag="s16")
    nc.tensor.matmul(pc2[:], lhsT=W("Wgf"), rhs=mT2b[:], start=True,
                     stop=True)
    c2Tf = rows.tile([128, 16], F32, tag="c2Tf")
    nc.scalar.activation(c2Tf[:], pc2[:], ACT.Tanh)

    ps_a2 = ps_st.tile([16, N], F32, tag="stat")
    a2q = []
    for cg in range(NCG):
        mlh = work.tile([128, 16], BF16, tag="mlh")
        nc.scalar.activation(mlh[:], csel(cg), ACT.Copy,
                             scale=c2Tf[:, cg:cg + 1])
        a2q.append((mlh, cg))
        if len(a2q) > 1:
            m0, c0 = a2q.pop(0)
            nc.tensor.matmul(ps_a2[:], lhsT=m0[:],
                             rhs=hpall[:, c0 * N:(c0 + 1) * N],
                             start=(c0 == 0), stop=False)
    m0, c0 = a2q.pop(0)
    nc.tensor.matmul(ps_a2[:], lhsT=m0[:], rhs=hpall[:, c0 * N:(c0 + 1) * N],
                     start=False, stop=True)
    wsum_row = rows.tile([16, N], F32, tag="wsum")
    nc.scalar.activation(wsum_row[:], ps_a2[:], ACT.Sigmoid)
    wsum_rowb = rows.tile([16, N], BF16, tag="wsumb")
    nc.vector.tensor_tensor(wsum_rowb[:], wsum_row[:], mask_row[:],
                            op=OP.mult)

    gcat = rows.tile([128, 16], F32, tag="gcat")
    for cg in range(NCG):
        bw = bcast_row(wsum_rowb, cg, N)
        sc3 = scr.tile([128, 512], BF16, tag="scr")
        nc.vector.scalar_tensor_tensor(
            sc3[:], hpall[:, cg * N:(cg + 1) * N], 1.0, bw[:],
            op0=OP.mult, op1=OP.mult, accum_out=gcat[:, cg:cg + 1])

    # ---- final MLP --------------------------------------------------------
    pcat = rows.tile([128, 16], BF16, tag="pcat")
    nc.vector.tensor_copy(pcat[:], gcat[:])
    p1b = bigtile(ps_mm)
    p1 = p1b[:, 0:128]
    nc.tensor.matmul(p1[:, 0:NPC], lhsT=W("Wl1a"), rhs=pcat[:, 0:NPC],
                     start=True, stop=False)
    nc.tensor.matmul(p1[:, 0:NPC], lhsT=W("Wl1b"), rhs=pcat[:, NPC:2 * NPC],
                     start=False, stop=True)
    o1 = rows.tile([128, NPC], BF16, tag="o1")
    nc.scalar.activation(o1[:], p1[:, 0:NPC], ACT.Relu, bias=Bc("bl1col")[:])
    p2b = bigtile(ps_mm)
    p2 = p2b[:, 0:128]
    nc.tensor.matmul(p2[0:64, 0:NPC], lhsT=W("Wl2"), rhs=o1[:], start=True,
                     stop=True)
    o2 = rows.tile([64, NPC], BF16, tag="o2")
    nc.scalar.activation(o2[:], p2[0:64, 0:NPC], ACT.Relu,
                         bias=Bc("bl2col")[0:64, :])
    p3b = bigtile(ps_mm)
    p3 = p3b[:, 0:128]
    nc.tensor.matmul(p3[0:2, 0:NPC], lhsT=W("Wl3")[0:64, :], rhs=o2[:],
                     start=True, stop=True)
    o3 = rows.tile([2, NPC], F32, tag="o3")
    nc.vector.tensor_scalar(o3[:], p3[0:2, 0:NPC], Bc("bl3col")[0:2, :],
                            None, op0=OP.add)
    nc.sync.dma_start(t_out.ap(), o3[:])
    ctx.close()


_NC_CACHE = {}


def _get_nc():
    key = (STAGE, DEBUG)
    if key not in _NC_CACHE:
        _NC_CACHE[key] = _build()
    return _NC_CACHE[key]


def kernel(**inputs):
    in_maps = _host_prep(inputs)
    nc = _get_nc()
    trace = bool(int(os.environ.get("KERNEL_TRACE", "0")))
    tmpdir = os.environ.get("KERNEL_TRACE_DIR") or None
    res = run_bass_kernel_spmd(nc, in_maps, core_ids=list(range(NCORES)),
                               trace=trace, tmpdir=tmpdir)
    out = np.empty((B, 2), np.float32)
    for c in range(NCORES):
        out[c * NPC:(c + 1) * NPC] = res.results[c]["out"].T
    kernel._last = res
    return out
